# revision 36
# baseline (speedup 1.0000x reference)
"""Trainium2 Bass kernel for the 3-metalayer forward-forward style MLP.

Distribution: the (10 labels x 512 batch) grid flattens to 5120 independent
rows; each of the 8 cores processes 640 rows (pure data parallelism, weights
replicated, no collectives).

Device-side algorithm (per core, rows R=640):
  - states kept feature-major [2048(part-chunks), R] in fp8e4 (scaled x128;
    safe since rows are L2-normalized so elements <= 1 -> <= 128 < 240 max)
  - weights quantized to fp8e4 with per-output-row power-of-2 scales
  - per linear term: DoubleRow PE matmuls (two 128-deep k-tiles per
    instruction at ~2x ALU rate), fp32 PSUM accumulate, ACT relu eviction
    with per-partition descale (1/(128*alpha_row)) + bias
  - 0.7/0.3 metalayer blend folded into host-prescaled weights/biases
    (relu positive homogeneity)
  - row L2 norms: 32*snew^2 in fp8e4 (scalar_tensor_tensor; x32 keeps small
    squares above the subnormal floor, max ~120 < 240) packed in 2-chunk
    pair tiles, reduced over partitions by DoubleRow ones-matmuls (half the
    reduce matmul count; the M=128 ones also broadcasts the row sums to
    every partition for free); inv = 128/n via DVE fast-reciprocal + ACT
    sqrt; goodness = sum(s^2)/2048 falls out of the same psums
  - t=0 terms with zero-state inputs are host-folded constants; the layer-1
    "pre" term (static overlay input) is computed once and reused all 3 steps
"""

import numpy as np
import ml_dtypes

import concourse.bass as bass
import concourse.tile as tile
from concourse import bacc, mybir
from concourse.bass_utils import run_bass_kernel_spmd

BF = mybir.dt.bfloat16
F8 = mybir.dt.float8e4
F32 = mybir.dt.float32
NPBF = ml_dtypes.bfloat16
NPF8 = ml_dtypes.float8_e4m3
DR = mybir.MatmulPerfMode.DoubleRow

N_CORES = 8
P = 128
D_IN = 784
D_IN_PAD = 896            # 7 * 128
KC1 = 7                   # k-chunks for the 784->2048 matmul
KC = 16                   # k-chunks for 2048-contraction matmuls
MC = 16                   # output-feature chunks (2048 / 128)
H = 2048
B = 512
NL = 10
ROWS = NL * B             # 5120
R = ROWS // N_CORES       # 640 rows per core
RH = 320                  # psum row-chunk (2 per core-row-block)
RAMP = 3                  # interleaved blocks at the start of gated passes
EPS = 1e-4
SSCALE = 128.0            # fp8 state scale (elements <= 1 after L2 norm)
SQSC = 32.0               # fp8 square scale: 32*snew^2 <= ~120 < 240

# bias/const column indices inside the packed [128, 12*16] bias tensor
B1PRE, B1POST, B1SELF, B2PRE, B2POST, B2SELF, B3PRE, B3SELF, C1, C2, C3, C3P = range(12)
NBIAS = 12
# weight-scale column groups in the [128, 8*16] wscales tensor
W1PRE, W1POST, W1SELF, W2PRE, W2POST, W2SELF, W3PRE, W3SELF = range(8)
NW = 8

_NC_CACHE = {}


def _build_nc():
    """Build the single-core Tile program (same NEFF for all 8 cores)."""
    nc = bacc.Bacc("TRN2", target_bir_lowering=False, debug=False,
                   num_devices=N_CORES)

    hx_d = nc.dram_tensor("hxn", [P, KC1, R], F8, kind="ExternalInput")
    w_d = {
        "w1pre": nc.dram_tensor("w1pre", [MC, P, KC1, P], F8, kind="ExternalInput"),
    }
    for name in ("w1post", "w1self", "w2pre", "w2post", "w2self", "w3pre", "w3self"):
        w_d[name] = nc.dram_tensor(name, [MC, P, KC, P], F8, kind="ExternalInput")
    bias_d = nc.dram_tensor("biases", [P, NBIAS * MC], F32, kind="ExternalInput")
    wsc_d = nc.dram_tensor("wscales", [P, NW * MC], F32, kind="ExternalInput")
    g_d = nc.dram_tensor("g", [1, R], F32, kind="ExternalOutput")

    with tile.TileContext(nc) as tc:
        with (
            tc.tile_pool(name="consts", bufs=1) as consts,
            tc.tile_pool(name="states", bufs=1) as states,
            tc.tile_pool(name="wpool", bufs=12) as wpool,
            tc.tile_pool(name="epool", bufs=6) as epool,
            tc.tile_pool(name="sqpool", bufs=6) as sqpool,
            tc.tile_pool(name="small", bufs=2) as small,
            tc.tile_pool(name="mmps", bufs=6, space="PSUM") as mmps,
            tc.tile_pool(name="redps", bufs=2, space="PSUM") as redps,
        ):
            # startup order: first hx chunk + first weight block must land
            # before anything else so the PE starts within ~1.5us
            hx = states.tile([P, KC1, R], F8, tag="hxn")
            nc.sync.dma_start(out=hx[:, 0, 0:RH], in_=hx_d[:, 0, 0:RH])
            bias_sb = consts.tile([P, NBIAS * MC], F32)
            wsc_sb = consts.tile([P, NW * MC], F32)
            w0 = wpool.tile([P, KC1, P], F8, tag="w", name="w1pre0")
            nc.sync.dma_start(out=w0[:], in_=w_d["w1pre"][0])
            nc.sync.dma_start(out=hx[:, 1, 0:RH], in_=hx_d[:, 1, 0:RH])
            nc.sync.dma_start(out=hx[:, 0, RH:R], in_=hx_d[:, 0, RH:R])
            nc.sync.dma_start(out=hx[:, 1, RH:R], in_=hx_d[:, 1, RH:R])
            nc.sync.dma_start(out=bias_sb[:], in_=bias_d[:])
            nc.sync.dma_start(out=wsc_sb[:], in_=wsc_d[:])
            for kc in range(2, KC1):
                nc.sync.dma_start(out=hx[:, kc, :], in_=hx_d[:, kc, :])
            # fp8 ones for the DoubleRow sum-of-squares reduction: M=128
            # ones-matmul both reduces over partitions AND broadcasts the
            # row sums to every partition for free
            ones_red = consts.tile([P, P], BF)
            nc.vector.memset(ones_red[:], 1.0)
            ones2 = consts.tile([P, 2, P], F8)
            nc.vector.memset(ones2[:], 1.0)
            gacc = consts.tile([1, R], F32)

            # warm the PE HAM clock gate while the initial DMAs are in
            # flight: dummy matmuls span >3.4us of PE activity, so the
            # real matmul stream starts at 2.4GHz instead of 1.2GHz
            warm_ps = mmps.tile([P, RH], F32, tag="mm", name="warm_ps")
            for _ in range(34):
                nc.tensor.matmul(warm_ps[:, :P], ones_red[:], ones_red[:],
                                 start=True, stop=True)
            At = states.tile([P, MC, R], BF, tag="A")
            s1 = states.tile([P, MC, R], F8, tag="s1")
            s2 = states.tile([P, MC, R], F8, tag="s2")
            s3 = states.tile([P, MC, R], F8, tag="s3")
            snew = states.tile([P, MC, R], BF, tag="snew")
            comb = states.tile([P, MC, R], BF, tag="comb")

            _red_uid = [0]

            def red_pair():
                _red_uid[0] += 1
                u = _red_uid[0]
                return (redps.tile([P, RH], F32, tag="red", name=f"red{u}a"),
                        redps.tile([P, RH], F32, tag="red", name=f"red{u}b"))

            def bias_ap(idx, mc):
                col = idx * MC + mc
                return bias_sb[:, col:col + 1]

            def wsc_ap(idx, mc):
                col = idx * MC + mc
                return wsc_sb[:, col:col + 1]

            def rsl(rh):
                return slice(rh * RH, (rh + 1) * RH)

            def mm_block(ps0, ps1, wt, src, kcn):
                """Accumulate one [2048->128] output block for both row
                chunks: DoubleRow over k-chunk pairs (plus a trailing single
                for odd kcn). kc-outer / rh-inner so the two matmuls sharing
                a stationary weight slice are adjacent."""
                npair = kcn // 2
                for kp in range(npair):
                    kc = 2 * kp
                    st = (kp == 0)
                    sp = (kc + 2 >= kcn)
                    nc.tensor.matmul(ps0[:], wt[:, kc:kc + 2, :],
                                     src[:, kc:kc + 2, rsl(0)],
                                     start=st, stop=sp, perf_mode=DR)
                    nc.tensor.matmul(ps1[:], wt[:, kc:kc + 2, :],
                                     src[:, kc:kc + 2, rsl(1)],
                                     start=st, stop=sp, perf_mode=DR)
                if kcn % 2:
                    kc = kcn - 1
                    nc.tensor.matmul(ps0[:], wt[:, kc, :], src[:, kc, rsl(0)],
                                     start=(kcn == 1), stop=True)
                    nc.tensor.matmul(ps1[:], wt[:, kc, :], src[:, kc, rsl(1)],
                                     start=(kcn == 1), stop=True)

            def term_pass(wname, kcn, src, evict, w0_tile=None, defer=2,
                          carry=(), ramp=False):
                """One linear term: stream weight blocks, accumulate psums,
                hand each [128, RH] psum chunk to `evict(mc, rh, ps)`.

                Evictions are emitted `defer` psum-groups late: the eviction
                chain (ACT relu -> DVE combine/square -> PE reduce-matmul)
                has ~1.5us of cross-engine latency, and emitting it inline
                makes the strict-FIFO PE queue stall on the reduce-matmul.

                The pass returns its last `defer` evictions instead of
                draining them: the CALLER either drains them inline (when
                the next pass's matmuls consume this pass's finale output)
                or hands them to the next pass as `carry` thunks, which run
                right after its first matmul block -- the ~2.2us of mains
                hides the drain's cross-engine chain, so the reduce-matmuls
                and the finale no longer serialize the pass boundary."""
                wd = w_d[wname]
                pending = []
                start_mc = 0
                if ramp:
                    # first RAMP blocks interleaved across 6 psums,
                    # consuming src k-chunks in ascending order: for passes
                    # whose src is being normalized by the immediately
                    # preceding finale, the PE consumes each chunk as the
                    # normalize mul produces it instead of stalling for all
                    # 16 chunks before block 0
                    wts, pss = [], []
                    for mc in range(RAMP):
                        wt = wpool.tile([P, kcn, P], F8, tag="w",
                                        name=f"rampw_{wname}_{mc}")
                        nc.sync.dma_start(out=wt[:], in_=wd[mc])
                        wts.append(wt)
                        pa = mmps.tile([P, RH], F32, tag="mm",
                                       name=f"rampp_{wname}_{mc}a")
                        pb = mmps.tile([P, RH], F32, tag="mm",
                                       name=f"rampp_{wname}_{mc}b")
                        pss.append((pa, pb))
                    for kp in range(kcn // 2):
                        kc = 2 * kp
                        st = (kp == 0)
                        sp = (kc + 2 >= kcn)
                        for mc in range(RAMP):
                            for rh in range(2):
                                nc.tensor.matmul(
                                    pss[mc][rh][:], wts[mc][:, kc:kc + 2, :],
                                    src[:, kc:kc + 2, rsl(rh)],
                                    start=st, stop=sp, perf_mode=DR)
                    for mc in range(RAMP):
                        pending.append((mc, 0, pss[mc][0]))
                        pending.append((mc, 1, pss[mc][1]))
                    start_mc = RAMP
                for mc in range(start_mc, MC):
                    if mc == 0 and w0_tile is not None:
                        wt = w0_tile
                    else:
                        wt = wpool.tile([P, kcn, P], F8, tag="w")
                        nc.sync.dma_start(out=wt[:], in_=wd[mc])
                    ps0 = mmps.tile([P, RH], F32, tag="mm")
                    ps1 = mmps.tile([P, RH], F32, tag="mm")
                    mm_block(ps0, ps1, wt, src, kcn)
                    if mc == start_mc:
                        for th in carry:
                            th()
                    pending.append((mc, 0, ps0))
                    pending.append((mc, 1, ps1))
                    while len(pending) > defer:
                        evict(*pending.pop(0))
                return evict, pending

            def finale_gated(tail, red, tgt):
                """Drain + finale for a pass whose output gates the very
                next pass: the rh0 inv chain is issued before the last rh1
                eviction drains, overlapping the two serial chains so the
                consumer's first (ramped) matmul unblocks ~1us sooner."""
                ev, pending = tail
                for it in pending[:-1]:
                    ev(*it)
                nr = small.tile([P, R], F32, tag="nr")
                inv = small.tile([P, R], F32, tag="inv")

                def chain(rh):
                    nc.vector.reciprocal_approx_fast(out=nr[:, rsl(rh)],
                                                     in_=red[rh][:])
                    nc.scalar.activation(
                        inv[:, rsl(rh)], nr[:, rsl(rh)],
                        mybir.ActivationFunctionType.Sqrt,
                        scale=SQSC * SSCALE * SSCALE)
                    for mc in range(2):
                        nc.vector.tensor_mul(tgt[:, mc, rsl(rh)],
                                             snew[:, mc, rsl(rh)],
                                             inv[:, rsl(rh)])

                chain(0)
                ev(*pending[-1])
                chain(1)
                for mc in range(2, MC):
                    nc.vector.tensor_mul(tgt[:, mc, :], snew[:, mc, :],
                                         inv[:])

            def drain(tail):
                ev, pending = tail
                for it in pending:
                    ev(*it)

            def mk_carry(tail, fin=None):
                ev, pending = tail
                ths = [lambda it=it: ev(*it) for it in pending]
                if fin is not None:
                    ths.append(fin)
                return ths

            _sqpair = {}

            def sq_and_reduce(mc, rh, red):
                """32*snew^2 in fp8e4 (x32 keeps the small squares above the
                fp8 subnormal floor; max ~120 < 240) into a 2-chunk pair
                tile; every odd mc issues one DoubleRow ones-matmul
                contracting both chunks -- half the reduce matmuls. red
                accumulates 32*sum(s^2), broadcast to all 128 partitions.
                On DVE (not ACT): keeps the ACT queue pure relu-evictions."""
                if mc % 2 == 0:
                    _sqpair[rh] = sqpool.tile([P, 2, RH], F8, tag="sq",
                                              name=f"sqp{rh}")
                t = _sqpair[rh]
                nc.vector.scalar_tensor_tensor(
                    t[:, mc % 2, :], snew[:, mc, rsl(rh)], SQSC,
                    snew[:, mc, rsl(rh)],
                    op0=mybir.AluOpType.mult, op1=mybir.AluOpType.mult)
                if mc % 2 == 1:
                    nc.tensor.matmul(red[rh][:], ones2[:], t[:],
                                     start=(mc == 1), stop=(mc == MC - 1),
                                     perf_mode=DR)

            def finale(red, tgt, goodness):
                """red[rh] holds 32*sum(s^2) per row, broadcast across all
                128 partitions. inv = SSCALE/sqrt(sum s^2): DVE fast
                reciprocal straight off the psum, then ACT sqrt with the
                scales folded into the input scale. eps dropped: n >= ~0.3
                always, so the relative effect is ~1e-4 (under the fp8
                noise floor). Normalize muls run full-R per mc so consumers'
                chunk k unblocks on mul #k."""
                if goodness:
                    for rh in range(2):
                        if goodness == "init":
                            nc.vector.tensor_copy(gacc[:, rsl(rh)],
                                                  red[rh][0:1, :])
                        else:
                            nc.vector.tensor_add(gacc[:, rsl(rh)],
                                                 gacc[:, rsl(rh)],
                                                 red[rh][0:1, :])
                if tgt is None:
                    return
                nr = small.tile([P, R], F32, tag="nr")
                inv = small.tile([P, R], F32, tag="inv")
                for rh in range(2):
                    nc.vector.reciprocal_approx_fast(out=nr[:, rsl(rh)],
                                                     in_=red[rh][:])
                    nc.scalar.activation(
                        inv[:, rsl(rh)], nr[:, rsl(rh)],
                        mybir.ActivationFunctionType.Sqrt,
                        scale=SQSC * SSCALE * SSCALE)
                    # the consumer's (ramped) first matmuls need chunks 0-1
                    # of this rh: emit them right after this rh's chain
                    # instead of behind both chains (full-R muls need all
                    # of inv)
                    for mc in range(2):
                        nc.vector.tensor_mul(tgt[:, mc, rsl(rh)],
                                             snew[:, mc, rsl(rh)],
                                             inv[:, rsl(rh)])
                for mc in range(2, MC):
                    nc.vector.tensor_mul(tgt[:, mc, :], snew[:, mc, :],
                                         inv[:])

            def evict_to(dst, bidx, widx):
                def ev(mc, rh, ps):
                    nc.scalar.activation(
                        dst[:, mc, rsl(rh)], ps[:],
                        mybir.ActivationFunctionType.Relu,
                        bias=bias_ap(bidx, mc), scale=wsc_ap(widx, mc))
                return ev

            def evict_add_comb(bidx, widx):
                def ev(mc, rh, ps):
                    e = epool.tile([P, RH], BF, tag="e")
                    nc.scalar.activation(
                        e[:], ps[:], mybir.ActivationFunctionType.Relu,
                        bias=bias_ap(bidx, mc), scale=wsc_ap(widx, mc))
                    nc.vector.tensor_add(comb[:, mc, rsl(rh)],
                                         e[:], comb[:, mc, rsl(rh)])
                return ev

            # ---- A = relu(hxn @ w1pre' + 0.7*b1pre), cached for all steps.
            # t0-n1 (snew = A + c1) is fused into the same pass so its
            # elementwise work overlaps the A matmuls chunk by chunk.
            red = red_pair()

            def ev_a(mc, rh, ps, red=red):
                nc.scalar.activation(
                    At[:, mc, rsl(rh)], ps[:],
                    mybir.ActivationFunctionType.Relu,
                    bias=bias_ap(B1PRE, mc), scale=wsc_ap(W1PRE, mc))
                nc.vector.tensor_scalar_add(
                    snew[:, mc, rsl(rh)], At[:, mc, rsl(rh)],
                    bias_ap(C1, mc))
                sq_and_reduce(mc, rh, red)

            # defer=4: the A pass produces chunks quickly, so the ~1.5us
            # eviction chain needs extra slack to stay hidden.
            # w2pre-t0 consumes s1 immediately -> drain + finale inline.
            finale_gated(term_pass("w1pre", KC1, hx, ev_a, w0_tile=w0,
                                    defer=2), red, s1)

            # ---- t0, n2 / n3: single pre-term + const.
            # t1-n1's post/self term passes are wedged between them: they
            # only need s2(t0)/s1(t0) and don't touch comb (the t0 updates
            # don't use it), so their matmuls fill t0's serial-chain tails.
            def ev_t0(red, cidx, bpre, widx):
                def ev(mc, rh, ps):
                    e = epool.tile([P, RH], BF, tag="e")
                    nc.scalar.activation(
                        e[:], ps[:], mybir.ActivationFunctionType.Relu,
                        bias=bias_ap(bpre, mc), scale=wsc_ap(widx, mc))
                    nc.vector.tensor_scalar_add(
                        snew[:, mc, rsl(rh)], e[:], bias_ap(cidx, mc))
                    sq_and_reduce(mc, rh, red)
                return ev

            red = red_pair()
            # w1post-t0 consumes s2 immediately -> drain + finale inline
            finale_gated(term_pass("w2pre", KC, s1,
                                    ev_t0(red, C2, B2PRE, W2PRE),
                                    ramp=True), red, s2)

            tail = term_pass("w1post", KC, s2,
                             evict_to(comb, B1POST, W1POST), ramp=True)
            tail = term_pass("w1self", KC, s1,
                             evict_add_comb(B1SELF, W1SELF),
                             carry=mk_carry(tail))

            red = red_pair()
            red_t0 = red
            tail = term_pass("w3pre", KC, s2, ev_t0(red, C3, B3PRE, W3PRE),
                             carry=mk_carry(tail))
            drain(tail)
            finale(red, s3, None)

            def n1_combine(last):
                red = red_pair()
                for mc in range(MC):
                    for rh in range(2):
                        nc.vector.tensor_add(snew[:, mc, rsl(rh)],
                                             At[:, mc, rsl(rh)],
                                             comb[:, mc, rsl(rh)])
                        sq_and_reduce(mc, rh, red)
                # s1's consumer (w2pre) is 2+ passes away: defer the finale
                # into the next pass
                return lambda: finale(red, s1, "init" if last else None)

            # ---- t1 / t2
            for t in (1, 2):
                last = (t == 2)
                # n1 = A + relu(s2@w1post'+b) + relu(s1@w1self'+b)
                if t == 2:
                    tail = term_pass("w1post", KC, s2,
                                     evict_to(comb, B1POST, W1POST),
                                     carry=carry_in)
                    tail = term_pass("w1self", KC, s1,
                                     evict_add_comb(B1SELF, W1SELF),
                                     carry=mk_carry(tail))
                    drain(tail)
                fin_n1 = n1_combine(last)

                # n2 = relu(s1new@w2pre') + relu(s3@w2post') + relu(s2@w2self')
                tail = term_pass("w2post", KC, s3,
                                 evict_to(comb, B2POST, W2POST),
                                 carry=[fin_n1])
                tail = term_pass("w2self", KC, s2,
                                 evict_add_comb(B2SELF, W2SELF),
                                 carry=mk_carry(tail))
                red = red_pair()

                def ev_n2(mc, rh, ps, red=red):
                    e = epool.tile([P, RH], BF, tag="e")
                    nc.scalar.activation(
                        e[:], ps[:], mybir.ActivationFunctionType.Relu,
                        bias=bias_ap(B2PRE, mc), scale=wsc_ap(W2PRE, mc))
                    nc.vector.tensor_add(snew[:, mc, rsl(rh)],
                                         e[:], comb[:, mc, rsl(rh)])
                    sq_and_reduce(mc, rh, red)

                tail = term_pass("w2pre", KC, s1, ev_n2,
                                 carry=mk_carry(tail))
                fin_n2 = (lambda red=red, g=("add" if last else None):
                          finale(red, s2, g))

                # n3 = relu(s2new@w3pre') + c3p + relu(s3@w3self')
                tail = term_pass("w3self", KC, s3,
                                 evict_to(comb, B3SELF, W3SELF),
                                 carry=mk_carry(tail, fin_n2))
                red = red_pair()

                def ev_n3(mc, rh, ps, red=red):
                    e = epool.tile([P, RH], BF, tag="e")
                    nc.scalar.activation(
                        e[:], ps[:], mybir.ActivationFunctionType.Relu,
                        bias=bias_ap(B3PRE, mc), scale=wsc_ap(W3PRE, mc))
                    nc.vector.scalar_tensor_tensor(
                        snew[:, mc, rsl(rh)], e[:], bias_ap(C3P, mc),
                        comb[:, mc, rsl(rh)],
                        op0=mybir.AluOpType.add, op1=mybir.AluOpType.add)
                    sq_and_reduce(mc, rh, red)

                tail = term_pass("w3pre", KC, s2, ev_n3,
                                 carry=mk_carry(tail))
                if last:
                    drain(tail)
                    finale(red, None, "add")
                else:
                    carry_in = mk_carry(
                        tail, (lambda red=red: finale(red, s3, None)))

            # ---- goodness out: gacc holds 32*sum(s^2); g = gacc/(32*2048)
            gout = consts.tile([1, R], F32, tag="gout")
            nc.scalar.mul(gout[:], gacc[:], 1.0 / (H * SQSC))
            nc.sync.dma_start(out=g_d[:], in_=gout[:])

    nc.compile()
    return nc


def _quant_weight(w, scale, kcn):
    """[2048, d_in] float32 -> ([MC, P, kcn, P] fp8e4 blocked for linear DMA,
    [128, 16] per-output-row descale columns).

    host_w[mc, p, kc, m] = alpha_row[mc*128+m] * scale * W[mc*128+m, kc*128+p]
    with alpha_row a power of 2 chosen so each row's absmax lands in
    (112, 224] (fp8e4 max normal 240). Descale col = 1/(128*alpha_row)."""
    w = np.asarray(w, dtype=np.float32) * scale
    din = w.shape[1]
    absmax = np.abs(w).max(axis=1)
    absmax = np.maximum(absmax, 1e-30)
    alpha = np.exp2(np.floor(np.log2(224.0 / absmax)))
    wq = w * alpha[:, None]
    if din < kcn * P:
        wq = np.pad(wq, ((0, 0), (0, kcn * P - din)))
    blk = wq.reshape(MC, P, kcn, P).transpose(0, 3, 2, 1)
    blk = np.ascontiguousarray(blk.astype(NPF8))
    descale = (1.0 / (SSCALE * alpha)).astype(np.float32).reshape(MC, P).T
    return blk, np.ascontiguousarray(descale)


def _col(v):
    """[2048] -> [128, 16] (partition-major bias layout)."""
    return np.asarray(v, dtype=np.float32).reshape(MC, P).T


def prepare_inputs(inputs):
    """Host prep: overlay+normalize Hx, quantize/block weights, pack biases.
    Returns (shared_map, per_core_hx list)."""
    x = np.asarray(inputs["x"], dtype=np.float32)
    mx = x.max()
    base = x.copy()
    base[:, :NL] = 0.0
    hx = np.tile(base[None, :, :], (NL, 1, 1))
    for l in range(NL):
        hx[l, :, l] = mx
    hx = hx.reshape(ROWS, D_IN)
    n = np.linalg.norm(hx, axis=1, keepdims=True)
    hxn = (hx / (n + EPS)) * SSCALE
    hxn = np.pad(hxn, ((0, 0), (0, D_IN_PAD - D_IN)))

    per_core_hx = []
    for c in range(N_CORES):
        h = hxn[c * R:(c + 1) * R].T            # [896, 640]
        h = h.reshape(KC1, P, R).transpose(1, 0, 2)
        per_core_hx.append(np.ascontiguousarray(h.astype(NPF8)))

    wspec = [
        ("w1pre", "w1_pre", 0.7, KC1, W1PRE),
        ("w1post", "w1_post", 0.7, KC, W1POST),
        ("w1self", "w1_self", 0.3, KC, W1SELF),
        ("w2pre", "w2_pre", 0.7, KC, W2PRE),
        ("w2post", "w2_post", 0.7, KC, W2POST),
        ("w2self", "w2_self", 0.3, KC, W2SELF),
        ("w3pre", "w3_pre", 0.7, KC, W3PRE),
        ("w3self", "w3_self", 0.3, KC, W3SELF),
    ]
    shared = {}
    wscales = np.empty((P, NW * MC), dtype=np.float32)
    for dname, iname, sc, kcn, widx in wspec:
        blk, desc = _quant_weight(inputs[iname], sc, kcn)
        shared[dname] = blk
        wscales[:, widx * MC:(widx + 1) * MC] = desc
    shared["wscales"] = np.ascontiguousarray(wscales)

    relu = lambda a: np.maximum(np.asarray(a, dtype=np.float32), 0.0)

    cols = np.empty((P, NBIAS * MC), dtype=np.float32)
    vals = {
        B1PRE: 0.7 * np.asarray(inputs["b1_pre"], np.float32),
        B1POST: 0.7 * np.asarray(inputs["b1_post"], np.float32),
        B1SELF: 0.3 * np.asarray(inputs["b1_self"], np.float32),
        B2PRE: 0.7 * np.asarray(inputs["b2_pre"], np.float32),
        B2POST: 0.7 * np.asarray(inputs["b2_post"], np.float32),
        B2SELF: 0.3 * np.asarray(inputs["b2_self"], np.float32),
        B3PRE: 0.7 * np.asarray(inputs["b3_pre"], np.float32),
        B3SELF: 0.3 * np.asarray(inputs["b3_self"], np.float32),
        C1: 0.7 * relu(inputs["b1_post"]) + 0.3 * relu(inputs["b1_self"]),
        C2: 0.7 * relu(inputs["b2_post"]) + 0.3 * relu(inputs["b2_self"]),
        C3: 0.7 * relu(inputs["b3_post"]) + 0.3 * relu(inputs["b3_self"]),
        C3P: 0.7 * relu(inputs["b3_post"]),
    }
    for idx, v in vals.items():
        cols[:, idx * MC:(idx + 1) * MC] = _col(v)
    shared["biases"] = np.ascontiguousarray(cols)

    return shared, per_core_hx


def run(inputs, trace=False):
    shared, per_core_hx = prepare_inputs(inputs)
    if "nc" not in _NC_CACHE:
        _NC_CACHE["nc"] = _build_nc()
    nc = _NC_CACHE["nc"]
    in_maps = [dict(shared, hxn=per_core_hx[c]) for c in range(N_CORES)]
    res = run_bass_kernel_spmd(nc, in_maps, core_ids=list(range(N_CORES)),
                               trace=trace)
    g = np.concatenate([res.results[c]["g"][0] for c in range(N_CORES)])
    out = g.reshape(NL, B).T.astype(np.float32)
    return np.ascontiguousarray(out), res


def kernel(**inputs):
    out, _ = run(inputs, trace=False)
    return out


# revision 38
# speedup vs baseline: 1.0021x; 1.0021x over previous
"""Trainium2 Bass kernel for the 3-metalayer forward-forward style MLP.

Distribution: the (10 labels x 512 batch) grid flattens to 5120 independent
rows; each of the 8 cores processes 640 rows (pure data parallelism, weights
replicated, no collectives).

Device-side algorithm (per core, rows R=640):
  - states kept feature-major [2048(part-chunks), R] in fp8e4 (scaled x128;
    safe since rows are L2-normalized so elements <= 1 -> <= 128 < 240 max)
  - weights quantized to fp8e4 with per-output-row power-of-2 scales
  - per linear term: DoubleRow PE matmuls (two 128-deep k-tiles per
    instruction at ~2x ALU rate), fp32 PSUM accumulate, ACT relu eviction
    with per-partition descale (1/(128*alpha_row)) + bias
  - 0.7/0.3 metalayer blend folded into host-prescaled weights/biases
    (relu positive homogeneity)
  - row L2 norms: 32*snew^2 in fp8e4 (scalar_tensor_tensor; x32 keeps small
    squares above the subnormal floor, max ~120 < 240) packed in 2-chunk
    pair tiles, reduced over partitions by DoubleRow ones-matmuls (half the
    reduce matmul count; the M=128 ones also broadcasts the row sums to
    every partition for free); inv = 128/n via DVE fast-reciprocal + ACT
    sqrt; goodness = sum(s^2)/2048 falls out of the same psums
  - t=0 terms with zero-state inputs are host-folded constants; the layer-1
    "pre" term (static overlay input) is computed once and reused all 3 steps
"""

import numpy as np
import ml_dtypes

import concourse.bass as bass
import concourse.tile as tile
from concourse import bacc, mybir
from concourse.bass_utils import run_bass_kernel_spmd

BF = mybir.dt.bfloat16
F8 = mybir.dt.float8e4
F32 = mybir.dt.float32
NPBF = ml_dtypes.bfloat16
NPF8 = ml_dtypes.float8_e4m3
DR = mybir.MatmulPerfMode.DoubleRow

N_CORES = 8
P = 128
D_IN = 784
D_IN_PAD = 896            # 7 * 128
KC1 = 7                   # k-chunks for the 784->2048 matmul
KC = 16                   # k-chunks for 2048-contraction matmuls
MC = 16                   # output-feature chunks (2048 / 128)
H = 2048
B = 512
NL = 10
ROWS = NL * B             # 5120
R = ROWS // N_CORES       # 640 rows per core
RH = 320                  # psum row-chunk (2 per core-row-block)
RAMP = 3                  # interleaved blocks at the start of gated passes
EPS = 1e-4
SSCALE = 128.0            # fp8 state scale (elements <= 1 after L2 norm)
SQSC = 32.0               # fp8 square scale: 32*snew^2 <= ~120 < 240

# bias/const column indices inside the packed [128, 12*16] bias tensor
B1PRE, B1POST, B1SELF, B2PRE, B2POST, B2SELF, B3PRE, B3SELF, C1, C2, C3, C3P = range(12)
NBIAS = 12
# weight-scale column groups in the [128, 8*16] wscales tensor
W1PRE, W1POST, W1SELF, W2PRE, W2POST, W2SELF, W3PRE, W3SELF = range(8)
NW = 8

_NC_CACHE = {}


def _build_nc():
    """Build the single-core Tile program (same NEFF for all 8 cores)."""
    nc = bacc.Bacc("TRN2", target_bir_lowering=False, debug=False,
                   num_devices=N_CORES)

    hx_d = nc.dram_tensor("hxn", [P, KC1, R], F8, kind="ExternalInput")
    w_d = {
        "w1pre": nc.dram_tensor("w1pre", [MC, P, KC1, P], F8, kind="ExternalInput"),
    }
    for name in ("w1post", "w1self", "w2pre", "w2post", "w2self", "w3pre", "w3self"):
        w_d[name] = nc.dram_tensor(name, [MC, P, KC, P], F8, kind="ExternalInput")
    bias_d = nc.dram_tensor("biases", [P, NBIAS * MC], F32, kind="ExternalInput")
    wsc_d = nc.dram_tensor("wscales", [P, NW * MC], F32, kind="ExternalInput")
    g_d = nc.dram_tensor("g", [1, R], F32, kind="ExternalOutput")

    with tile.TileContext(nc) as tc:
        with (
            tc.tile_pool(name="consts", bufs=1) as consts,
            tc.tile_pool(name="states", bufs=1) as states,
            tc.tile_pool(name="wpool", bufs=12) as wpool,
            tc.tile_pool(name="epool", bufs=6) as epool,
            tc.tile_pool(name="sqpool", bufs=6) as sqpool,
            tc.tile_pool(name="small", bufs=2) as small,
            tc.tile_pool(name="mmps", bufs=6, space="PSUM") as mmps,
            tc.tile_pool(name="redps", bufs=2, space="PSUM") as redps,
        ):
            # startup order: first hx chunk + first weight block must land
            # before anything else so the PE starts within ~1.5us
            hx = states.tile([P, KC1, R], F8, tag="hxn")
            nc.sync.dma_start(out=hx[:, 0, :], in_=hx_d[:, 0, :])
            bias_sb = consts.tile([P, NBIAS * MC], F32)
            wsc_sb = consts.tile([P, NW * MC], F32)
            w0 = wpool.tile([P, KC1, P], F8, tag="w", name="w1pre0")
            nc.sync.dma_start(out=w0[:], in_=w_d["w1pre"][0])
            nc.sync.dma_start(out=hx[:, 1, :], in_=hx_d[:, 1, :])
            nc.sync.dma_start(out=bias_sb[:], in_=bias_d[:])
            nc.sync.dma_start(out=wsc_sb[:], in_=wsc_d[:])
            for kc in range(2, KC1):
                nc.sync.dma_start(out=hx[:, kc, :], in_=hx_d[:, kc, :])
            # fp8 ones for the DoubleRow sum-of-squares reduction: M=128
            # ones-matmul both reduces over partitions AND broadcasts the
            # row sums to every partition for free
            ones_red = consts.tile([P, P], BF)
            nc.vector.memset(ones_red[:], 1.0)
            ones2 = consts.tile([P, 2, P], F8)
            nc.vector.memset(ones2[:], 1.0)
            gacc = consts.tile([1, R], F32)

            # warm the PE HAM clock gate while the initial DMAs are in
            # flight: dummy matmuls span >3.4us of PE activity, so the
            # real matmul stream starts at 2.4GHz instead of 1.2GHz
            warm_ps = mmps.tile([P, RH], F32, tag="mm", name="warm_ps")
            for _ in range(48):
                nc.tensor.matmul(warm_ps[:, :P], ones_red[:], ones_red[:],
                                 start=True, stop=True)
            At = states.tile([P, MC, R], BF, tag="A")
            s1 = states.tile([P, MC, R], F8, tag="s1")
            s2 = states.tile([P, MC, R], F8, tag="s2")
            s3 = states.tile([P, MC, R], F8, tag="s3")
            snew = states.tile([P, MC, R], BF, tag="snew")
            comb = states.tile([P, MC, R], BF, tag="comb")

            _red_uid = [0]

            def red_pair():
                _red_uid[0] += 1
                u = _red_uid[0]
                return (redps.tile([P, RH], F32, tag="red", name=f"red{u}a"),
                        redps.tile([P, RH], F32, tag="red", name=f"red{u}b"))

            def bias_ap(idx, mc):
                col = idx * MC + mc
                return bias_sb[:, col:col + 1]

            def wsc_ap(idx, mc):
                col = idx * MC + mc
                return wsc_sb[:, col:col + 1]

            def rsl(rh):
                return slice(rh * RH, (rh + 1) * RH)

            def mm_block(ps0, ps1, wt, src, kcn):
                """Accumulate one [2048->128] output block for both row
                chunks: DoubleRow over k-chunk pairs (plus a trailing single
                for odd kcn). kc-outer / rh-inner so the two matmuls sharing
                a stationary weight slice are adjacent."""
                npair = kcn // 2
                for kp in range(npair):
                    kc = 2 * kp
                    st = (kp == 0)
                    sp = (kc + 2 >= kcn)
                    nc.tensor.matmul(ps0[:], wt[:, kc:kc + 2, :],
                                     src[:, kc:kc + 2, rsl(0)],
                                     start=st, stop=sp, perf_mode=DR)
                    nc.tensor.matmul(ps1[:], wt[:, kc:kc + 2, :],
                                     src[:, kc:kc + 2, rsl(1)],
                                     start=st, stop=sp, perf_mode=DR)
                if kcn % 2:
                    kc = kcn - 1
                    nc.tensor.matmul(ps0[:], wt[:, kc, :], src[:, kc, rsl(0)],
                                     start=(kcn == 1), stop=True)
                    nc.tensor.matmul(ps1[:], wt[:, kc, :], src[:, kc, rsl(1)],
                                     start=(kcn == 1), stop=True)

            def term_pass(wname, kcn, src, evict, w0_tile=None, defer=2,
                          carry=(), ramp=False):
                """One linear term: stream weight blocks, accumulate psums,
                hand each [128, RH] psum chunk to `evict(mc, rh, ps)`.

                Evictions are emitted `defer` psum-groups late: the eviction
                chain (ACT relu -> DVE combine/square -> PE reduce-matmul)
                has ~1.5us of cross-engine latency, and emitting it inline
                makes the strict-FIFO PE queue stall on the reduce-matmul.

                The pass returns its last `defer` evictions instead of
                draining them: the CALLER either drains them inline (when
                the next pass's matmuls consume this pass's finale output)
                or hands them to the next pass as `carry` thunks, which run
                right after its first matmul block -- the ~2.2us of mains
                hides the drain's cross-engine chain, so the reduce-matmuls
                and the finale no longer serialize the pass boundary."""
                wd = w_d[wname]
                pending = []
                start_mc = 0
                if ramp:
                    # first RAMP blocks interleaved across 6 psums,
                    # consuming src k-chunks in ascending order: for passes
                    # whose src is being normalized by the immediately
                    # preceding finale, the PE consumes each chunk as the
                    # normalize mul produces it instead of stalling for all
                    # 16 chunks before block 0
                    wts, pss = [], []
                    for mc in range(RAMP):
                        wt = wpool.tile([P, kcn, P], F8, tag="w",
                                        name=f"rampw_{wname}_{mc}")
                        nc.sync.dma_start(out=wt[:], in_=wd[mc])
                        wts.append(wt)
                        pa = mmps.tile([P, RH], F32, tag="mm",
                                       name=f"rampp_{wname}_{mc}a")
                        pb = mmps.tile([P, RH], F32, tag="mm",
                                       name=f"rampp_{wname}_{mc}b")
                        pss.append((pa, pb))
                    for kp in range(kcn // 2):
                        kc = 2 * kp
                        st = (kp == 0)
                        sp = (kc + 2 >= kcn)
                        for mc in range(RAMP):
                            for rh in range(2):
                                nc.tensor.matmul(
                                    pss[mc][rh][:], wts[mc][:, kc:kc + 2, :],
                                    src[:, kc:kc + 2, rsl(rh)],
                                    start=st, stop=sp, perf_mode=DR)
                    for mc in range(RAMP):
                        pending.append((mc, 0, pss[mc][0]))
                        pending.append((mc, 1, pss[mc][1]))
                    start_mc = RAMP
                for mc in range(start_mc, MC):
                    if mc == 0 and w0_tile is not None:
                        wt = w0_tile
                    else:
                        wt = wpool.tile([P, kcn, P], F8, tag="w")
                        nc.sync.dma_start(out=wt[:], in_=wd[mc])
                    ps0 = mmps.tile([P, RH], F32, tag="mm")
                    ps1 = mmps.tile([P, RH], F32, tag="mm")
                    mm_block(ps0, ps1, wt, src, kcn)
                    if mc == start_mc:
                        for th in carry:
                            th()
                    pending.append((mc, 0, ps0))
                    pending.append((mc, 1, ps1))
                    while len(pending) > defer:
                        evict(*pending.pop(0))
                return evict, pending

            def finale_gated(tail, red, tgt):
                """Drain + finale for a pass whose output gates the very
                next pass: the rh0 inv chain is issued before the last rh1
                eviction drains, overlapping the two serial chains so the
                consumer's first (ramped) matmul unblocks ~1us sooner."""
                ev, pending = tail
                for it in pending[:-1]:
                    ev(*it)
                nr = small.tile([P, R], F32, tag="nr")
                inv = small.tile([P, R], F32, tag="inv")

                def chain(rh):
                    nc.vector.reciprocal_approx_fast(out=nr[:, rsl(rh)],
                                                     in_=red[rh][:])
                    nc.scalar.activation(
                        inv[:, rsl(rh)], nr[:, rsl(rh)],
                        mybir.ActivationFunctionType.Sqrt,
                        scale=SQSC * SSCALE * SSCALE)
                    for mc in range(2):
                        nc.vector.tensor_mul(tgt[:, mc, rsl(rh)],
                                             snew[:, mc, rsl(rh)],
                                             inv[:, rsl(rh)])

                chain(0)
                ev(*pending[-1])
                chain(1)
                for mc in range(2, MC):
                    nc.vector.tensor_mul(tgt[:, mc, :], snew[:, mc, :],
                                         inv[:])

            def drain(tail):
                ev, pending = tail
                for it in pending:
                    ev(*it)

            def mk_carry(tail, fin=None):
                ev, pending = tail
                ths = [lambda it=it: ev(*it) for it in pending]
                if fin is not None:
                    ths.append(fin)
                return ths

            _sqpair = {}

            def sq_and_reduce(mc, rh, red):
                """32*snew^2 in fp8e4 (x32 keeps the small squares above the
                fp8 subnormal floor; max ~120 < 240) into a 2-chunk pair
                tile; every odd mc issues one DoubleRow ones-matmul
                contracting both chunks -- half the reduce matmuls. red
                accumulates 32*sum(s^2), broadcast to all 128 partitions.
                On DVE (not ACT): keeps the ACT queue pure relu-evictions."""
                if mc % 2 == 0:
                    _sqpair[rh] = sqpool.tile([P, 2, RH], F8, tag="sq",
                                              name=f"sqp{rh}")
                t = _sqpair[rh]
                nc.vector.scalar_tensor_tensor(
                    t[:, mc % 2, :], snew[:, mc, rsl(rh)], SQSC,
                    snew[:, mc, rsl(rh)],
                    op0=mybir.AluOpType.mult, op1=mybir.AluOpType.mult)
                if mc % 2 == 1:
                    nc.tensor.matmul(red[rh][:], ones2[:], t[:],
                                     start=(mc == 1), stop=(mc == MC - 1),
                                     perf_mode=DR)

            def finale(red, tgt, goodness):
                """red[rh] holds 32*sum(s^2) per row, broadcast across all
                128 partitions. inv = SSCALE/sqrt(sum s^2): DVE fast
                reciprocal straight off the psum, then ACT sqrt with the
                scales folded into the input scale. eps dropped: n >= ~0.3
                always, so the relative effect is ~1e-4 (under the fp8
                noise floor). Normalize muls run full-R per mc so consumers'
                chunk k unblocks on mul #k."""
                if goodness:
                    for rh in range(2):
                        if goodness == "init":
                            nc.vector.tensor_copy(gacc[:, rsl(rh)],
                                                  red[rh][0:1, :])
                        else:
                            nc.vector.tensor_add(gacc[:, rsl(rh)],
                                                 gacc[:, rsl(rh)],
                                                 red[rh][0:1, :])
                if tgt is None:
                    return
                nr = small.tile([P, R], F32, tag="nr")
                inv = small.tile([P, R], F32, tag="inv")
                for rh in range(2):
                    nc.vector.reciprocal_approx_fast(out=nr[:, rsl(rh)],
                                                     in_=red[rh][:])
                    nc.scalar.activation(
                        inv[:, rsl(rh)], nr[:, rsl(rh)],
                        mybir.ActivationFunctionType.Sqrt,
                        scale=SQSC * SSCALE * SSCALE)
                    # the consumer's (ramped) first matmuls need chunks 0-1
                    # of this rh: emit them right after this rh's chain
                    # instead of behind both chains (full-R muls need all
                    # of inv)
                    for mc in range(2):
                        nc.vector.tensor_mul(tgt[:, mc, rsl(rh)],
                                             snew[:, mc, rsl(rh)],
                                             inv[:, rsl(rh)])
                for mc in range(2, MC):
                    nc.vector.tensor_mul(tgt[:, mc, :], snew[:, mc, :],
                                         inv[:])

            def evict_to(dst, bidx, widx):
                def ev(mc, rh, ps):
                    nc.scalar.activation(
                        dst[:, mc, rsl(rh)], ps[:],
                        mybir.ActivationFunctionType.Relu,
                        bias=bias_ap(bidx, mc), scale=wsc_ap(widx, mc))
                return ev

            def evict_add_comb(bidx, widx):
                def ev(mc, rh, ps):
                    e = epool.tile([P, RH], BF, tag="e")
                    nc.scalar.activation(
                        e[:], ps[:], mybir.ActivationFunctionType.Relu,
                        bias=bias_ap(bidx, mc), scale=wsc_ap(widx, mc))
                    nc.vector.tensor_add(comb[:, mc, rsl(rh)],
                                         e[:], comb[:, mc, rsl(rh)])
                return ev

            # ---- A = relu(hxn @ w1pre' + 0.7*b1pre), cached for all steps.
            # t0-n1 (snew = A + c1) is fused into the same pass so its
            # elementwise work overlaps the A matmuls chunk by chunk.
            red = red_pair()

            def ev_a(mc, rh, ps, red=red):
                nc.scalar.activation(
                    At[:, mc, rsl(rh)], ps[:],
                    mybir.ActivationFunctionType.Relu,
                    bias=bias_ap(B1PRE, mc), scale=wsc_ap(W1PRE, mc))
                nc.vector.tensor_scalar_add(
                    snew[:, mc, rsl(rh)], At[:, mc, rsl(rh)],
                    bias_ap(C1, mc))
                sq_and_reduce(mc, rh, red)

            # defer=4: the A pass produces chunks quickly, so the ~1.5us
            # eviction chain needs extra slack to stay hidden.
            # w2pre-t0 consumes s1 immediately -> drain + finale inline.
            finale_gated(term_pass("w1pre", KC1, hx, ev_a, w0_tile=w0,
                                    defer=2), red, s1)

            # ---- t0, n2 / n3: single pre-term + const.
            # t1-n1's post/self term passes are wedged between them: they
            # only need s2(t0)/s1(t0) and don't touch comb (the t0 updates
            # don't use it), so their matmuls fill t0's serial-chain tails.
            def ev_t0(red, cidx, bpre, widx):
                def ev(mc, rh, ps):
                    e = epool.tile([P, RH], BF, tag="e")
                    nc.scalar.activation(
                        e[:], ps[:], mybir.ActivationFunctionType.Relu,
                        bias=bias_ap(bpre, mc), scale=wsc_ap(widx, mc))
                    nc.vector.tensor_scalar_add(
                        snew[:, mc, rsl(rh)], e[:], bias_ap(cidx, mc))
                    sq_and_reduce(mc, rh, red)
                return ev

            red = red_pair()
            # w1post-t0 consumes s2 immediately -> drain + finale inline
            finale_gated(term_pass("w2pre", KC, s1,
                                    ev_t0(red, C2, B2PRE, W2PRE),
                                    ramp=True), red, s2)

            tail = term_pass("w1post", KC, s2,
                             evict_to(comb, B1POST, W1POST), ramp=True)
            tail = term_pass("w1self", KC, s1,
                             evict_add_comb(B1SELF, W1SELF),
                             carry=mk_carry(tail))

            red = red_pair()
            red_t0 = red
            tail = term_pass("w3pre", KC, s2, ev_t0(red, C3, B3PRE, W3PRE),
                             carry=mk_carry(tail), defer=3)
            drain(tail)
            finale(red, s3, None)

            def n1_combine(last):
                red = red_pair()
                for mc in range(MC):
                    for rh in range(2):
                        nc.vector.tensor_add(snew[:, mc, rsl(rh)],
                                             At[:, mc, rsl(rh)],
                                             comb[:, mc, rsl(rh)])
                        sq_and_reduce(mc, rh, red)
                # s1's consumer (w2pre) is 2+ passes away: defer the finale
                # into the next pass
                return lambda: finale(red, s1, "init" if last else None)

            # ---- t1 / t2
            for t in (1, 2):
                last = (t == 2)
                # n1 = A + relu(s2@w1post'+b) + relu(s1@w1self'+b)
                if t == 2:
                    tail = term_pass("w1post", KC, s2,
                                     evict_to(comb, B1POST, W1POST),
                                     carry=carry_in)
                    tail = term_pass("w1self", KC, s1,
                                     evict_add_comb(B1SELF, W1SELF),
                                     carry=mk_carry(tail))
                    drain(tail)
                fin_n1 = n1_combine(last)

                # n2 = relu(s1new@w2pre') + relu(s3@w2post') + relu(s2@w2self')
                tail = term_pass("w2post", KC, s3,
                                 evict_to(comb, B2POST, W2POST),
                                 carry=[fin_n1])
                tail = term_pass("w2self", KC, s2,
                                 evict_add_comb(B2SELF, W2SELF),
                                 carry=mk_carry(tail))
                red = red_pair()

                def ev_n2(mc, rh, ps, red=red):
                    e = epool.tile([P, RH], BF, tag="e")
                    nc.scalar.activation(
                        e[:], ps[:], mybir.ActivationFunctionType.Relu,
                        bias=bias_ap(B2PRE, mc), scale=wsc_ap(W2PRE, mc))
                    nc.vector.tensor_add(snew[:, mc, rsl(rh)],
                                         e[:], comb[:, mc, rsl(rh)])
                    sq_and_reduce(mc, rh, red)

                tail = term_pass("w2pre", KC, s1, ev_n2,
                                 carry=mk_carry(tail), defer=3)
                fin_n2 = (lambda red=red, g=("add" if last else None):
                          finale(red, s2, g))

                # n3 = relu(s2new@w3pre') + c3p + relu(s3@w3self')
                tail = term_pass("w3self", KC, s3,
                                 evict_to(comb, B3SELF, W3SELF),
                                 carry=mk_carry(tail, fin_n2))
                red = red_pair()

                def ev_n3(mc, rh, ps, red=red):
                    e = epool.tile([P, RH], BF, tag="e")
                    nc.scalar.activation(
                        e[:], ps[:], mybir.ActivationFunctionType.Relu,
                        bias=bias_ap(B3PRE, mc), scale=wsc_ap(W3PRE, mc))
                    nc.vector.scalar_tensor_tensor(
                        snew[:, mc, rsl(rh)], e[:], bias_ap(C3P, mc),
                        comb[:, mc, rsl(rh)],
                        op0=mybir.AluOpType.add, op1=mybir.AluOpType.add)
                    sq_and_reduce(mc, rh, red)

                tail = term_pass("w3pre", KC, s2, ev_n3,
                                 carry=mk_carry(tail), defer=3)
                if last:
                    drain(tail)
                    finale(red, None, "add")
                else:
                    carry_in = mk_carry(
                        tail, (lambda red=red: finale(red, s3, None)))

            # ---- goodness out: gacc holds 32*sum(s^2); g = gacc/(32*2048)
            gout = consts.tile([1, R], F32, tag="gout")
            nc.scalar.mul(gout[:], gacc[:], 1.0 / (H * SQSC))
            nc.sync.dma_start(out=g_d[:], in_=gout[:])

    nc.compile()
    return nc


def _quant_weight(w, scale, kcn):
    """[2048, d_in] float32 -> ([MC, P, kcn, P] fp8e4 blocked for linear DMA,
    [128, 16] per-output-row descale columns).

    host_w[mc, p, kc, m] = alpha_row[mc*128+m] * scale * W[mc*128+m, kc*128+p]
    with alpha_row a power of 2 chosen so each row's absmax lands in
    (112, 224] (fp8e4 max normal 240). Descale col = 1/(128*alpha_row)."""
    w = np.asarray(w, dtype=np.float32) * scale
    din = w.shape[1]
    absmax = np.abs(w).max(axis=1)
    absmax = np.maximum(absmax, 1e-30)
    alpha = np.exp2(np.floor(np.log2(224.0 / absmax)))
    wq = w * alpha[:, None]
    if din < kcn * P:
        wq = np.pad(wq, ((0, 0), (0, kcn * P - din)))
    blk = wq.reshape(MC, P, kcn, P).transpose(0, 3, 2, 1)
    blk = np.ascontiguousarray(blk.astype(NPF8))
    descale = (1.0 / (SSCALE * alpha)).astype(np.float32).reshape(MC, P).T
    return blk, np.ascontiguousarray(descale)


def _col(v):
    """[2048] -> [128, 16] (partition-major bias layout)."""
    return np.asarray(v, dtype=np.float32).reshape(MC, P).T


def prepare_inputs(inputs):
    """Host prep: overlay+normalize Hx, quantize/block weights, pack biases.
    Returns (shared_map, per_core_hx list)."""
    x = np.asarray(inputs["x"], dtype=np.float32)
    mx = x.max()
    base = x.copy()
    base[:, :NL] = 0.0
    hx = np.tile(base[None, :, :], (NL, 1, 1))
    for l in range(NL):
        hx[l, :, l] = mx
    hx = hx.reshape(ROWS, D_IN)
    n = np.linalg.norm(hx, axis=1, keepdims=True)
    hxn = (hx / (n + EPS)) * SSCALE
    hxn = np.pad(hxn, ((0, 0), (0, D_IN_PAD - D_IN)))

    per_core_hx = []
    for c in range(N_CORES):
        h = hxn[c * R:(c + 1) * R].T            # [896, 640]
        h = h.reshape(KC1, P, R).transpose(1, 0, 2)
        per_core_hx.append(np.ascontiguousarray(h.astype(NPF8)))

    wspec = [
        ("w1pre", "w1_pre", 0.7, KC1, W1PRE),
        ("w1post", "w1_post", 0.7, KC, W1POST),
        ("w1self", "w1_self", 0.3, KC, W1SELF),
        ("w2pre", "w2_pre", 0.7, KC, W2PRE),
        ("w2post", "w2_post", 0.7, KC, W2POST),
        ("w2self", "w2_self", 0.3, KC, W2SELF),
        ("w3pre", "w3_pre", 0.7, KC, W3PRE),
        ("w3self", "w3_self", 0.3, KC, W3SELF),
    ]
    shared = {}
    wscales = np.empty((P, NW * MC), dtype=np.float32)
    for dname, iname, sc, kcn, widx in wspec:
        blk, desc = _quant_weight(inputs[iname], sc, kcn)
        shared[dname] = blk
        wscales[:, widx * MC:(widx + 1) * MC] = desc
    shared["wscales"] = np.ascontiguousarray(wscales)

    relu = lambda a: np.maximum(np.asarray(a, dtype=np.float32), 0.0)

    cols = np.empty((P, NBIAS * MC), dtype=np.float32)
    vals = {
        B1PRE: 0.7 * np.asarray(inputs["b1_pre"], np.float32),
        B1POST: 0.7 * np.asarray(inputs["b1_post"], np.float32),
        B1SELF: 0.3 * np.asarray(inputs["b1_self"], np.float32),
        B2PRE: 0.7 * np.asarray(inputs["b2_pre"], np.float32),
        B2POST: 0.7 * np.asarray(inputs["b2_post"], np.float32),
        B2SELF: 0.3 * np.asarray(inputs["b2_self"], np.float32),
        B3PRE: 0.7 * np.asarray(inputs["b3_pre"], np.float32),
        B3SELF: 0.3 * np.asarray(inputs["b3_self"], np.float32),
        C1: 0.7 * relu(inputs["b1_post"]) + 0.3 * relu(inputs["b1_self"]),
        C2: 0.7 * relu(inputs["b2_post"]) + 0.3 * relu(inputs["b2_self"]),
        C3: 0.7 * relu(inputs["b3_post"]) + 0.3 * relu(inputs["b3_self"]),
        C3P: 0.7 * relu(inputs["b3_post"]),
    }
    for idx, v in vals.items():
        cols[:, idx * MC:(idx + 1) * MC] = _col(v)
    shared["biases"] = np.ascontiguousarray(cols)

    return shared, per_core_hx


def run(inputs, trace=False):
    shared, per_core_hx = prepare_inputs(inputs)
    if "nc" not in _NC_CACHE:
        _NC_CACHE["nc"] = _build_nc()
    nc = _NC_CACHE["nc"]
    in_maps = [dict(shared, hxn=per_core_hx[c]) for c in range(N_CORES)]
    res = run_bass_kernel_spmd(nc, in_maps, core_ids=list(range(N_CORES)),
                               trace=trace)
    g = np.concatenate([res.results[c]["g"][0] for c in range(N_CORES)])
    out = g.reshape(NL, B).T.astype(np.float32)
    return np.ascontiguousarray(out), res


def kernel(**inputs):
    out, _ = run(inputs, trace=False)
    return out


# revision 39
# speedup vs baseline: 1.0033x; 1.0011x over previous
"""Trainium2 Bass kernel for the 3-metalayer forward-forward style MLP.

Distribution: the (10 labels x 512 batch) grid flattens to 5120 independent
rows; each of the 8 cores processes 640 rows (pure data parallelism, weights
replicated, no collectives).

Device-side algorithm (per core, rows R=640):
  - states kept feature-major [2048(part-chunks), R] in fp8e4 (scaled x128;
    safe since rows are L2-normalized so elements <= 1 -> <= 128 < 240 max)
  - weights quantized to fp8e4 with per-output-row power-of-2 scales
  - per linear term: DoubleRow PE matmuls (two 128-deep k-tiles per
    instruction at ~2x ALU rate), fp32 PSUM accumulate, ACT relu eviction
    with per-partition descale (1/(128*alpha_row)) + bias
  - 0.7/0.3 metalayer blend folded into host-prescaled weights/biases
    (relu positive homogeneity)
  - row L2 norms: 32*snew^2 in fp8e4 (scalar_tensor_tensor; x32 keeps small
    squares above the subnormal floor, max ~120 < 240) packed in 2-chunk
    pair tiles, reduced over partitions by DoubleRow ones-matmuls (half the
    reduce matmul count; the M=128 ones also broadcasts the row sums to
    every partition for free); inv = 128/n via DVE fast-reciprocal + ACT
    sqrt; goodness = sum(s^2)/2048 falls out of the same psums
  - t=0 terms with zero-state inputs are host-folded constants; the layer-1
    "pre" term (static overlay input) is computed once and reused all 3 steps
"""

import numpy as np
import ml_dtypes

import concourse.bass as bass
import concourse.tile as tile
from concourse import bacc, mybir
from concourse.bass_utils import run_bass_kernel_spmd

BF = mybir.dt.bfloat16
F8 = mybir.dt.float8e4
F32 = mybir.dt.float32
NPBF = ml_dtypes.bfloat16
NPF8 = ml_dtypes.float8_e4m3
DR = mybir.MatmulPerfMode.DoubleRow

N_CORES = 8
P = 128
D_IN = 784
D_IN_PAD = 896            # 7 * 128
KC1 = 7                   # k-chunks for the 784->2048 matmul
KC = 16                   # k-chunks for 2048-contraction matmuls
MC = 16                   # output-feature chunks (2048 / 128)
H = 2048
B = 512
NL = 10
ROWS = NL * B             # 5120
R = ROWS // N_CORES       # 640 rows per core
RH = 320                  # psum row-chunk (2 per core-row-block)
RAMP = 3                  # interleaved blocks at the start of gated passes
EPS = 1e-4
SSCALE = 128.0            # fp8 state scale (elements <= 1 after L2 norm)
SQSC = 32.0               # fp8 square scale: 32*snew^2 <= ~120 < 240

# bias/const column indices inside the packed [128, 12*16] bias tensor
B1PRE, B1POST, B1SELF, B2PRE, B2POST, B2SELF, B3PRE, B3SELF, C1, C2, C3, C3P = range(12)
NBIAS = 12
# weight-scale column groups in the [128, 8*16] wscales tensor
W1PRE, W1POST, W1SELF, W2PRE, W2POST, W2SELF, W3PRE, W3SELF = range(8)
NW = 8

_NC_CACHE = {}


def _build_nc():
    """Build the single-core Tile program (same NEFF for all 8 cores)."""
    nc = bacc.Bacc("TRN2", target_bir_lowering=False, debug=False,
                   num_devices=N_CORES)

    hx_d = nc.dram_tensor("hxn", [P, KC1, R], F8, kind="ExternalInput")
    w_d = {
        "w1pre": nc.dram_tensor("w1pre", [MC, P, KC1, P], F8, kind="ExternalInput"),
    }
    for name in ("w1post", "w1self", "w2pre", "w2post", "w2self", "w3pre", "w3self"):
        w_d[name] = nc.dram_tensor(name, [MC, P, KC, P], F8, kind="ExternalInput")
    bias_d = nc.dram_tensor("biases", [P, NBIAS * MC], F32, kind="ExternalInput")
    wsc_d = nc.dram_tensor("wscales", [P, NW * MC], F32, kind="ExternalInput")
    g_d = nc.dram_tensor("g", [1, R], F32, kind="ExternalOutput")

    with tile.TileContext(nc) as tc:
        with (
            tc.tile_pool(name="consts", bufs=1) as consts,
            tc.tile_pool(name="states", bufs=1) as states,
            tc.tile_pool(name="wpool", bufs=12) as wpool,
            tc.tile_pool(name="epool", bufs=6) as epool,
            tc.tile_pool(name="sqpool", bufs=6) as sqpool,
            tc.tile_pool(name="small", bufs=2) as small,
            tc.tile_pool(name="mmps", bufs=6, space="PSUM") as mmps,
            tc.tile_pool(name="redps", bufs=2, space="PSUM") as redps,
        ):
            # startup order: first hx chunk + first weight block must land
            # before anything else so the PE starts within ~1.5us
            hx = states.tile([P, KC1, R], F8, tag="hxn")
            nc.sync.dma_start(out=hx[:, 0, :], in_=hx_d[:, 0, :])
            bias_sb = consts.tile([P, NBIAS * MC], F32)
            wsc_sb = consts.tile([P, NW * MC], F32)
            w0 = wpool.tile([P, KC1, P], F8, tag="w", name="w1pre0")
            nc.sync.dma_start(out=w0[:], in_=w_d["w1pre"][0])
            nc.sync.dma_start(out=hx[:, 1, :], in_=hx_d[:, 1, :])
            nc.sync.dma_start(out=bias_sb[:], in_=bias_d[:])
            nc.sync.dma_start(out=wsc_sb[:], in_=wsc_d[:])
            for kc in range(2, KC1):
                nc.sync.dma_start(out=hx[:, kc, :], in_=hx_d[:, kc, :])
            # fp8 ones for the DoubleRow sum-of-squares reduction: M=128
            # ones-matmul both reduces over partitions AND broadcasts the
            # row sums to every partition for free
            ones_red = consts.tile([P, P], BF)
            nc.vector.memset(ones_red[:], 1.0)
            ones2 = consts.tile([P, 2, P], F8)
            nc.vector.memset(ones2[:], 1.0)
            gacc = consts.tile([1, R], F32)

            # warm the PE HAM clock gate while the initial DMAs are in
            # flight: dummy matmuls span >3.4us of PE activity, so the
            # real matmul stream starts at 2.4GHz instead of 1.2GHz
            warm_ps = mmps.tile([P, RH], F32, tag="mm", name="warm_ps")
            for _ in range(48):
                nc.tensor.matmul(warm_ps[:, :P], ones_red[:], ones_red[:],
                                 start=True, stop=True)
            At = states.tile([P, MC, R], BF, tag="A")
            s1 = states.tile([P, MC, R], F8, tag="s1")
            s2 = states.tile([P, MC, R], F8, tag="s2")
            s3 = states.tile([P, MC, R], F8, tag="s3")
            snew = states.tile([P, MC, R], BF, tag="snew")
            comb = states.tile([P, MC, R], BF, tag="comb")

            _red_uid = [0]

            def red_pair():
                _red_uid[0] += 1
                u = _red_uid[0]
                return (redps.tile([P, RH], F32, tag="red", name=f"red{u}a"),
                        redps.tile([P, RH], F32, tag="red", name=f"red{u}b"))

            def bias_ap(idx, mc):
                col = idx * MC + mc
                return bias_sb[:, col:col + 1]

            def wsc_ap(idx, mc):
                col = idx * MC + mc
                return wsc_sb[:, col:col + 1]

            def rsl(rh):
                return slice(rh * RH, (rh + 1) * RH)

            def mm_block(ps0, ps1, wt, src, kcn):
                """Accumulate one [2048->128] output block for both row
                chunks: DoubleRow over k-chunk pairs (plus a trailing single
                for odd kcn). kc-outer / rh-inner so the two matmuls sharing
                a stationary weight slice are adjacent."""
                npair = kcn // 2
                for kp in range(npair):
                    kc = 2 * kp
                    st = (kp == 0)
                    sp = (kc + 2 >= kcn)
                    nc.tensor.matmul(ps0[:], wt[:, kc:kc + 2, :],
                                     src[:, kc:kc + 2, rsl(0)],
                                     start=st, stop=sp, perf_mode=DR)
                    nc.tensor.matmul(ps1[:], wt[:, kc:kc + 2, :],
                                     src[:, kc:kc + 2, rsl(1)],
                                     start=st, stop=sp, perf_mode=DR)
                if kcn % 2:
                    kc = kcn - 1
                    nc.tensor.matmul(ps0[:], wt[:, kc, :], src[:, kc, rsl(0)],
                                     start=(kcn == 1), stop=True)
                    nc.tensor.matmul(ps1[:], wt[:, kc, :], src[:, kc, rsl(1)],
                                     start=(kcn == 1), stop=True)

            def term_pass(wname, kcn, src, evict, w0_tile=None, defer=2,
                          carry=(), ramp=False):
                """One linear term: stream weight blocks, accumulate psums,
                hand each [128, RH] psum chunk to `evict(mc, rh, ps)`.

                Evictions are emitted `defer` psum-groups late: the eviction
                chain (ACT relu -> DVE combine/square -> PE reduce-matmul)
                has ~1.5us of cross-engine latency, and emitting it inline
                makes the strict-FIFO PE queue stall on the reduce-matmul.

                The pass returns its last `defer` evictions instead of
                draining them: the CALLER either drains them inline (when
                the next pass's matmuls consume this pass's finale output)
                or hands them to the next pass as `carry` thunks, which run
                right after its first matmul block -- the ~2.2us of mains
                hides the drain's cross-engine chain, so the reduce-matmuls
                and the finale no longer serialize the pass boundary."""
                wd = w_d[wname]
                pending = []
                start_mc = 0
                if ramp:
                    # first RAMP blocks interleaved across 6 psums,
                    # consuming src k-chunks in ascending order: for passes
                    # whose src is being normalized by the immediately
                    # preceding finale, the PE consumes each chunk as the
                    # normalize mul produces it instead of stalling for all
                    # 16 chunks before block 0
                    wts, pss = [], []
                    for mc in range(RAMP):
                        wt = wpool.tile([P, kcn, P], F8, tag="w",
                                        name=f"rampw_{wname}_{mc}")
                        nc.sync.dma_start(out=wt[:], in_=wd[mc])
                        wts.append(wt)
                        pa = mmps.tile([P, RH], F32, tag="mm",
                                       name=f"rampp_{wname}_{mc}a")
                        pb = mmps.tile([P, RH], F32, tag="mm",
                                       name=f"rampp_{wname}_{mc}b")
                        pss.append((pa, pb))
                    for kp in range(kcn // 2):
                        kc = 2 * kp
                        st = (kp == 0)
                        sp = (kc + 2 >= kcn)
                        for mc in range(RAMP):
                            for rh in range(2):
                                nc.tensor.matmul(
                                    pss[mc][rh][:], wts[mc][:, kc:kc + 2, :],
                                    src[:, kc:kc + 2, rsl(rh)],
                                    start=st, stop=sp, perf_mode=DR)
                    for mc in range(RAMP):
                        pending.append((mc, 0, pss[mc][0]))
                        pending.append((mc, 1, pss[mc][1]))
                    start_mc = RAMP
                for mc in range(start_mc, MC):
                    if mc == 0 and w0_tile is not None:
                        wt = w0_tile
                    else:
                        wt = wpool.tile([P, kcn, P], F8, tag="w")
                        nc.sync.dma_start(out=wt[:], in_=wd[mc])
                    ps0 = mmps.tile([P, RH], F32, tag="mm")
                    ps1 = mmps.tile([P, RH], F32, tag="mm")
                    mm_block(ps0, ps1, wt, src, kcn)
                    if mc == start_mc:
                        for th in carry:
                            th()
                    pending.append((mc, 0, ps0))
                    pending.append((mc, 1, ps1))
                    while len(pending) > defer:
                        evict(*pending.pop(0))
                return evict, pending

            def finale_gated(tail, red, tgt):
                """Drain + finale for a pass whose output gates the very
                next pass: the rh0 inv chain is issued before the last rh1
                eviction drains, overlapping the two serial chains so the
                consumer's first (ramped) matmul unblocks ~1us sooner."""
                ev, pending = tail
                for it in pending[:-1]:
                    ev(*it)
                nr = small.tile([P, R], F32, tag="nr")
                inv = small.tile([P, R], F32, tag="inv")

                def chain(rh):
                    nc.vector.reciprocal_approx_fast(out=nr[:, rsl(rh)],
                                                     in_=red[rh][:])
                    nc.scalar.activation(
                        inv[:, rsl(rh)], nr[:, rsl(rh)],
                        mybir.ActivationFunctionType.Sqrt,
                        scale=SQSC * SSCALE * SSCALE)
                    for mc in range(2):
                        nc.vector.tensor_mul(tgt[:, mc, rsl(rh)],
                                             snew[:, mc, rsl(rh)],
                                             inv[:, rsl(rh)])

                chain(0)
                ev(*pending[-1])
                chain(1)
                for mc in range(2, MC):
                    nc.vector.tensor_mul(tgt[:, mc, :], snew[:, mc, :],
                                         inv[:])

            def drain(tail):
                ev, pending = tail
                for it in pending:
                    ev(*it)

            def mk_carry(tail, fin=None):
                ev, pending = tail
                ths = [lambda it=it: ev(*it) for it in pending]
                if fin is not None:
                    ths.append(fin)
                return ths

            _sqpair = {}

            def sq_and_reduce(mc, rh, red):
                """32*snew^2 in fp8e4 (x32 keeps the small squares above the
                fp8 subnormal floor; max ~120 < 240) into a 2-chunk pair
                tile; every odd mc issues one DoubleRow ones-matmul
                contracting both chunks -- half the reduce matmuls. red
                accumulates 32*sum(s^2), broadcast to all 128 partitions.
                On DVE (not ACT): keeps the ACT queue pure relu-evictions."""
                if mc % 2 == 0:
                    _sqpair[rh] = sqpool.tile([P, 2, RH], F8, tag="sq",
                                              name=f"sqp{rh}")
                t = _sqpair[rh]
                nc.vector.scalar_tensor_tensor(
                    t[:, mc % 2, :], snew[:, mc, rsl(rh)], SQSC,
                    snew[:, mc, rsl(rh)],
                    op0=mybir.AluOpType.mult, op1=mybir.AluOpType.mult)
                if mc % 2 == 1:
                    nc.tensor.matmul(red[rh][:], ones2[:], t[:],
                                     start=(mc == 1), stop=(mc == MC - 1),
                                     perf_mode=DR)

            def finale(red, tgt, goodness):
                """red[rh] holds 32*sum(s^2) per row, broadcast across all
                128 partitions. inv = SSCALE/sqrt(sum s^2): DVE fast
                reciprocal straight off the psum, then ACT sqrt with the
                scales folded into the input scale. eps dropped: n >= ~0.3
                always, so the relative effect is ~1e-4 (under the fp8
                noise floor). Normalize muls run full-R per mc so consumers'
                chunk k unblocks on mul #k."""
                if goodness:
                    for rh in range(2):
                        if goodness == "init":
                            nc.vector.tensor_copy(gacc[:, rsl(rh)],
                                                  red[rh][0:1, :])
                        else:
                            nc.vector.tensor_add(gacc[:, rsl(rh)],
                                                 gacc[:, rsl(rh)],
                                                 red[rh][0:1, :])
                if tgt is None:
                    return
                nr = small.tile([P, R], F32, tag="nr")
                inv = small.tile([P, R], F32, tag="inv")
                for rh in range(2):
                    nc.vector.reciprocal_approx_fast(out=nr[:, rsl(rh)],
                                                     in_=red[rh][:])
                    nc.scalar.activation(
                        inv[:, rsl(rh)], nr[:, rsl(rh)],
                        mybir.ActivationFunctionType.Sqrt,
                        scale=SQSC * SSCALE * SSCALE)
                    # the consumer's (ramped) first matmuls need chunks 0-1
                    # of this rh: emit them right after this rh's chain
                    # instead of behind both chains (full-R muls need all
                    # of inv)
                    for mc in range(2):
                        nc.vector.tensor_mul(tgt[:, mc, rsl(rh)],
                                             snew[:, mc, rsl(rh)],
                                             inv[:, rsl(rh)])
                for mc in range(2, MC):
                    nc.vector.tensor_mul(tgt[:, mc, :], snew[:, mc, :],
                                         inv[:])

            def evict_to(dst, bidx, widx):
                def ev(mc, rh, ps):
                    nc.scalar.activation(
                        dst[:, mc, rsl(rh)], ps[:],
                        mybir.ActivationFunctionType.Relu,
                        bias=bias_ap(bidx, mc), scale=wsc_ap(widx, mc))
                return ev

            def evict_add_comb(bidx, widx):
                def ev(mc, rh, ps):
                    e = epool.tile([P, RH], BF, tag="e")
                    nc.scalar.activation(
                        e[:], ps[:], mybir.ActivationFunctionType.Relu,
                        bias=bias_ap(bidx, mc), scale=wsc_ap(widx, mc))
                    nc.vector.tensor_add(comb[:, mc, rsl(rh)],
                                         e[:], comb[:, mc, rsl(rh)])
                return ev

            # ---- A = relu(hxn @ w1pre' + 0.7*b1pre), cached for all steps.
            # t0-n1 (snew = A + c1) is fused into the same pass so its
            # elementwise work overlaps the A matmuls chunk by chunk.
            red = red_pair()

            def ev_a(mc, rh, ps, red=red):
                nc.scalar.activation(
                    At[:, mc, rsl(rh)], ps[:],
                    mybir.ActivationFunctionType.Relu,
                    bias=bias_ap(B1PRE, mc), scale=wsc_ap(W1PRE, mc))
                nc.vector.tensor_scalar_add(
                    snew[:, mc, rsl(rh)], At[:, mc, rsl(rh)],
                    bias_ap(C1, mc))
                sq_and_reduce(mc, rh, red)

            # defer=4: the A pass produces chunks quickly, so the ~1.5us
            # eviction chain needs extra slack to stay hidden.
            # w2pre-t0 consumes s1 immediately -> drain + finale inline.
            finale_gated(term_pass("w1pre", KC1, hx, ev_a, w0_tile=w0,
                                    defer=3), red, s1)

            # ---- t0, n2 / n3: single pre-term + const.
            # t1-n1's post/self term passes are wedged between them: they
            # only need s2(t0)/s1(t0) and don't touch comb (the t0 updates
            # don't use it), so their matmuls fill t0's serial-chain tails.
            def ev_t0(red, cidx, bpre, widx):
                def ev(mc, rh, ps):
                    e = epool.tile([P, RH], BF, tag="e")
                    nc.scalar.activation(
                        e[:], ps[:], mybir.ActivationFunctionType.Relu,
                        bias=bias_ap(bpre, mc), scale=wsc_ap(widx, mc))
                    nc.vector.tensor_scalar_add(
                        snew[:, mc, rsl(rh)], e[:], bias_ap(cidx, mc))
                    sq_and_reduce(mc, rh, red)
                return ev

            red = red_pair()
            # w1post-t0 consumes s2 immediately -> drain + finale inline
            finale_gated(term_pass("w2pre", KC, s1,
                                    ev_t0(red, C2, B2PRE, W2PRE),
                                    ramp=True), red, s2)

            tail = term_pass("w1post", KC, s2,
                             evict_to(comb, B1POST, W1POST), ramp=True)
            tail = term_pass("w1self", KC, s1,
                             evict_add_comb(B1SELF, W1SELF),
                             carry=mk_carry(tail))

            red = red_pair()
            red_t0 = red
            tail = term_pass("w3pre", KC, s2, ev_t0(red, C3, B3PRE, W3PRE),
                             carry=mk_carry(tail), defer=3)
            drain(tail)
            finale(red, s3, None)

            def n1_combine(last):
                red = red_pair()
                for mc in range(MC):
                    for rh in range(2):
                        nc.vector.tensor_add(snew[:, mc, rsl(rh)],
                                             At[:, mc, rsl(rh)],
                                             comb[:, mc, rsl(rh)])
                        sq_and_reduce(mc, rh, red)
                # s1's consumer (w2pre) is 2+ passes away: defer the finale
                # into the next pass
                return lambda: finale(red, s1, "init" if last else None)

            # ---- t1 / t2
            for t in (1, 2):
                last = (t == 2)
                # n1 = A + relu(s2@w1post'+b) + relu(s1@w1self'+b)
                if t == 2:
                    tail = term_pass("w1post", KC, s2,
                                     evict_to(comb, B1POST, W1POST),
                                     carry=carry_in)
                    tail = term_pass("w1self", KC, s1,
                                     evict_add_comb(B1SELF, W1SELF),
                                     carry=mk_carry(tail))
                    drain(tail)
                fin_n1 = n1_combine(last)

                # n2 = relu(s1new@w2pre') + relu(s3@w2post') + relu(s2@w2self')
                tail = term_pass("w2post", KC, s3,
                                 evict_to(comb, B2POST, W2POST),
                                 carry=[fin_n1])
                tail = term_pass("w2self", KC, s2,
                                 evict_add_comb(B2SELF, W2SELF),
                                 carry=mk_carry(tail))
                red = red_pair()

                def ev_n2(mc, rh, ps, red=red):
                    e = epool.tile([P, RH], BF, tag="e")
                    nc.scalar.activation(
                        e[:], ps[:], mybir.ActivationFunctionType.Relu,
                        bias=bias_ap(B2PRE, mc), scale=wsc_ap(W2PRE, mc))
                    nc.vector.tensor_add(snew[:, mc, rsl(rh)],
                                         e[:], comb[:, mc, rsl(rh)])
                    sq_and_reduce(mc, rh, red)

                tail = term_pass("w2pre", KC, s1, ev_n2,
                                 carry=mk_carry(tail), defer=3)
                fin_n2 = (lambda red=red, g=("add" if last else None):
                          finale(red, s2, g))

                # n3 = relu(s2new@w3pre') + c3p + relu(s3@w3self')
                tail = term_pass("w3self", KC, s3,
                                 evict_to(comb, B3SELF, W3SELF),
                                 carry=mk_carry(tail, fin_n2))
                red = red_pair()

                def ev_n3(mc, rh, ps, red=red):
                    e = epool.tile([P, RH], BF, tag="e")
                    nc.scalar.activation(
                        e[:], ps[:], mybir.ActivationFunctionType.Relu,
                        bias=bias_ap(B3PRE, mc), scale=wsc_ap(W3PRE, mc))
                    nc.vector.scalar_tensor_tensor(
                        snew[:, mc, rsl(rh)], e[:], bias_ap(C3P, mc),
                        comb[:, mc, rsl(rh)],
                        op0=mybir.AluOpType.add, op1=mybir.AluOpType.add)
                    sq_and_reduce(mc, rh, red)

                tail = term_pass("w3pre", KC, s2, ev_n3,
                                 carry=mk_carry(tail), defer=3)
                if last:
                    drain(tail)
                    finale(red, None, "add")
                else:
                    carry_in = mk_carry(
                        tail, (lambda red=red: finale(red, s3, None)))

            # ---- goodness out: gacc holds 32*sum(s^2); g = gacc/(32*2048)
            gout = consts.tile([1, R], F32, tag="gout")
            nc.scalar.mul(gout[:], gacc[:], 1.0 / (H * SQSC))
            nc.sync.dma_start(out=g_d[:], in_=gout[:])

    nc.compile()
    return nc


def _quant_weight(w, scale, kcn):
    """[2048, d_in] float32 -> ([MC, P, kcn, P] fp8e4 blocked for linear DMA,
    [128, 16] per-output-row descale columns).

    host_w[mc, p, kc, m] = alpha_row[mc*128+m] * scale * W[mc*128+m, kc*128+p]
    with alpha_row a power of 2 chosen so each row's absmax lands in
    (112, 224] (fp8e4 max normal 240). Descale col = 1/(128*alpha_row)."""
    w = np.asarray(w, dtype=np.float32) * scale
    din = w.shape[1]
    absmax = np.abs(w).max(axis=1)
    absmax = np.maximum(absmax, 1e-30)
    alpha = np.exp2(np.floor(np.log2(224.0 / absmax)))
    wq = w * alpha[:, None]
    if din < kcn * P:
        wq = np.pad(wq, ((0, 0), (0, kcn * P - din)))
    blk = wq.reshape(MC, P, kcn, P).transpose(0, 3, 2, 1)
    blk = np.ascontiguousarray(blk.astype(NPF8))
    descale = (1.0 / (SSCALE * alpha)).astype(np.float32).reshape(MC, P).T
    return blk, np.ascontiguousarray(descale)


def _col(v):
    """[2048] -> [128, 16] (partition-major bias layout)."""
    return np.asarray(v, dtype=np.float32).reshape(MC, P).T


def prepare_inputs(inputs):
    """Host prep: overlay+normalize Hx, quantize/block weights, pack biases.
    Returns (shared_map, per_core_hx list)."""
    x = np.asarray(inputs["x"], dtype=np.float32)
    mx = x.max()
    base = x.copy()
    base[:, :NL] = 0.0
    hx = np.tile(base[None, :, :], (NL, 1, 1))
    for l in range(NL):
        hx[l, :, l] = mx
    hx = hx.reshape(ROWS, D_IN)
    n = np.linalg.norm(hx, axis=1, keepdims=True)
    hxn = (hx / (n + EPS)) * SSCALE
    hxn = np.pad(hxn, ((0, 0), (0, D_IN_PAD - D_IN)))

    per_core_hx = []
    for c in range(N_CORES):
        h = hxn[c * R:(c + 1) * R].T            # [896, 640]
        h = h.reshape(KC1, P, R).transpose(1, 0, 2)
        per_core_hx.append(np.ascontiguousarray(h.astype(NPF8)))

    wspec = [
        ("w1pre", "w1_pre", 0.7, KC1, W1PRE),
        ("w1post", "w1_post", 0.7, KC, W1POST),
        ("w1self", "w1_self", 0.3, KC, W1SELF),
        ("w2pre", "w2_pre", 0.7, KC, W2PRE),
        ("w2post", "w2_post", 0.7, KC, W2POST),
        ("w2self", "w2_self", 0.3, KC, W2SELF),
        ("w3pre", "w3_pre", 0.7, KC, W3PRE),
        ("w3self", "w3_self", 0.3, KC, W3SELF),
    ]
    shared = {}
    wscales = np.empty((P, NW * MC), dtype=np.float32)
    for dname, iname, sc, kcn, widx in wspec:
        blk, desc = _quant_weight(inputs[iname], sc, kcn)
        shared[dname] = blk
        wscales[:, widx * MC:(widx + 1) * MC] = desc
    shared["wscales"] = np.ascontiguousarray(wscales)

    relu = lambda a: np.maximum(np.asarray(a, dtype=np.float32), 0.0)

    cols = np.empty((P, NBIAS * MC), dtype=np.float32)
    vals = {
        B1PRE: 0.7 * np.asarray(inputs["b1_pre"], np.float32),
        B1POST: 0.7 * np.asarray(inputs["b1_post"], np.float32),
        B1SELF: 0.3 * np.asarray(inputs["b1_self"], np.float32),
        B2PRE: 0.7 * np.asarray(inputs["b2_pre"], np.float32),
        B2POST: 0.7 * np.asarray(inputs["b2_post"], np.float32),
        B2SELF: 0.3 * np.asarray(inputs["b2_self"], np.float32),
        B3PRE: 0.7 * np.asarray(inputs["b3_pre"], np.float32),
        B3SELF: 0.3 * np.asarray(inputs["b3_self"], np.float32),
        C1: 0.7 * relu(inputs["b1_post"]) + 0.3 * relu(inputs["b1_self"]),
        C2: 0.7 * relu(inputs["b2_post"]) + 0.3 * relu(inputs["b2_self"]),
        C3: 0.7 * relu(inputs["b3_post"]) + 0.3 * relu(inputs["b3_self"]),
        C3P: 0.7 * relu(inputs["b3_post"]),
    }
    for idx, v in vals.items():
        cols[:, idx * MC:(idx + 1) * MC] = _col(v)
    shared["biases"] = np.ascontiguousarray(cols)

    return shared, per_core_hx


def run(inputs, trace=False):
    shared, per_core_hx = prepare_inputs(inputs)
    if "nc" not in _NC_CACHE:
        _NC_CACHE["nc"] = _build_nc()
    nc = _NC_CACHE["nc"]
    in_maps = [dict(shared, hxn=per_core_hx[c]) for c in range(N_CORES)]
    res = run_bass_kernel_spmd(nc, in_maps, core_ids=list(range(N_CORES)),
                               trace=trace)
    g = np.concatenate([res.results[c]["g"][0] for c in range(N_CORES)])
    out = g.reshape(NL, B).T.astype(np.float32)
    return np.ascontiguousarray(out), res


def kernel(**inputs):
    out, _ = run(inputs, trace=False)
    return out


# revision 40
# speedup vs baseline: 1.0040x; 1.0008x over previous
"""Trainium2 Bass kernel for the 3-metalayer forward-forward style MLP.

Distribution: the (10 labels x 512 batch) grid flattens to 5120 independent
rows; each of the 8 cores processes 640 rows (pure data parallelism, weights
replicated, no collectives).

Device-side algorithm (per core, rows R=640):
  - states kept feature-major [2048(part-chunks), R] in fp8e4 (scaled x128;
    safe since rows are L2-normalized so elements <= 1 -> <= 128 < 240 max)
  - weights quantized to fp8e4 with per-output-row power-of-2 scales
  - per linear term: DoubleRow PE matmuls (two 128-deep k-tiles per
    instruction at ~2x ALU rate), fp32 PSUM accumulate, ACT relu eviction
    with per-partition descale (1/(128*alpha_row)) + bias
  - 0.7/0.3 metalayer blend folded into host-prescaled weights/biases
    (relu positive homogeneity)
  - row L2 norms: 32*snew^2 in fp8e4 (scalar_tensor_tensor; x32 keeps small
    squares above the subnormal floor, max ~120 < 240) packed in 2-chunk
    pair tiles, reduced over partitions by DoubleRow ones-matmuls (half the
    reduce matmul count; the M=128 ones also broadcasts the row sums to
    every partition for free); inv = 128/n via DVE fast-reciprocal + ACT
    sqrt; goodness = sum(s^2)/2048 falls out of the same psums
  - t=0 terms with zero-state inputs are host-folded constants; the layer-1
    "pre" term (static overlay input) is computed once and reused all 3 steps
"""

import numpy as np
import ml_dtypes

import concourse.bass as bass
import concourse.tile as tile
from concourse import bacc, mybir
from concourse.bass_utils import run_bass_kernel_spmd

BF = mybir.dt.bfloat16
F8 = mybir.dt.float8e4
F32 = mybir.dt.float32
NPBF = ml_dtypes.bfloat16
NPF8 = ml_dtypes.float8_e4m3
DR = mybir.MatmulPerfMode.DoubleRow

N_CORES = 8
P = 128
D_IN = 784
D_IN_PAD = 896            # 7 * 128
KC1 = 7                   # k-chunks for the 784->2048 matmul
KC = 16                   # k-chunks for 2048-contraction matmuls
MC = 16                   # output-feature chunks (2048 / 128)
H = 2048
B = 512
NL = 10
ROWS = NL * B             # 5120
R = ROWS // N_CORES       # 640 rows per core
RH = 320                  # psum row-chunk (2 per core-row-block)
RAMP = 3                  # interleaved blocks at the start of gated passes
EPS = 1e-4
SSCALE = 128.0            # fp8 state scale (elements <= 1 after L2 norm)
SQSC = 32.0               # fp8 square scale: 32*snew^2 <= ~120 < 240

# bias/const column indices inside the packed [128, 12*16] bias tensor
B1PRE, B1POST, B1SELF, B2PRE, B2POST, B2SELF, B3PRE, B3SELF, C1, C2, C3, C3P = range(12)
NBIAS = 12
# weight-scale column groups in the [128, 8*16] wscales tensor
W1PRE, W1POST, W1SELF, W2PRE, W2POST, W2SELF, W3PRE, W3SELF = range(8)
NW = 8

_NC_CACHE = {}


def _build_nc():
    """Build the single-core Tile program (same NEFF for all 8 cores)."""
    nc = bacc.Bacc("TRN2", target_bir_lowering=False, debug=False,
                   num_devices=N_CORES)

    hx_d = nc.dram_tensor("hxn", [P, KC1, R], F8, kind="ExternalInput")
    w_d = {
        "w1pre": nc.dram_tensor("w1pre", [MC, P, KC1, P], F8, kind="ExternalInput"),
    }
    for name in ("w1post", "w1self", "w2pre", "w2post", "w2self", "w3pre", "w3self"):
        w_d[name] = nc.dram_tensor(name, [MC, P, KC, P], F8, kind="ExternalInput")
    bias_d = nc.dram_tensor("biases", [P, NBIAS * MC], F32, kind="ExternalInput")
    wsc_d = nc.dram_tensor("wscales", [P, NW * MC], F32, kind="ExternalInput")
    g_d = nc.dram_tensor("g", [1, R], F32, kind="ExternalOutput")

    with tile.TileContext(nc) as tc:
        with (
            tc.tile_pool(name="consts", bufs=1) as consts,
            tc.tile_pool(name="states", bufs=1) as states,
            tc.tile_pool(name="wpool", bufs=12) as wpool,
            tc.tile_pool(name="epool", bufs=6) as epool,
            tc.tile_pool(name="sqpool", bufs=6) as sqpool,
            tc.tile_pool(name="small", bufs=2) as small,
            tc.tile_pool(name="mmps", bufs=6, space="PSUM") as mmps,
            tc.tile_pool(name="redps", bufs=2, space="PSUM") as redps,
        ):
            # startup order: first hx chunk + first weight block must land
            # before anything else so the PE starts within ~1.5us
            hx = states.tile([P, KC1, R], F8, tag="hxn")
            nc.sync.dma_start(out=hx[:, 0, :], in_=hx_d[:, 0, :])
            bias_sb = consts.tile([P, NBIAS * MC], F32)
            wsc_sb = consts.tile([P, NW * MC], F32)
            w0 = wpool.tile([P, KC1, P], F8, tag="w", name="w1pre0")
            nc.sync.dma_start(out=w0[:], in_=w_d["w1pre"][0])
            nc.sync.dma_start(out=hx[:, 1, :], in_=hx_d[:, 1, :])
            nc.sync.dma_start(out=bias_sb[:], in_=bias_d[:])
            nc.sync.dma_start(out=wsc_sb[:], in_=wsc_d[:])
            for kc in range(2, KC1):
                nc.sync.dma_start(out=hx[:, kc, :], in_=hx_d[:, kc, :])
            # fp8 ones for the DoubleRow sum-of-squares reduction: M=128
            # ones-matmul both reduces over partitions AND broadcasts the
            # row sums to every partition for free
            ones_red = consts.tile([P, P], BF)
            nc.vector.memset(ones_red[:], 1.0)
            ones2 = consts.tile([P, 2, P], F8)
            nc.vector.memset(ones2[:], 1.0)
            gacc = consts.tile([1, R], F32)

            # warm the PE HAM clock gate while the initial DMAs are in
            # flight: dummy matmuls span >3.4us of PE activity, so the
            # real matmul stream starts at 2.4GHz instead of 1.2GHz
            warm_ps = mmps.tile([P, RH], F32, tag="mm", name="warm_ps")
            for _ in range(48):
                nc.tensor.matmul(warm_ps[:, :P], ones_red[:], ones_red[:],
                                 start=True, stop=True)
            At = states.tile([P, MC, R], BF, tag="A")
            s1 = states.tile([P, MC, R], F8, tag="s1")
            s2 = states.tile([P, MC, R], F8, tag="s2")
            s3 = states.tile([P, MC, R], F8, tag="s3")
            snew = states.tile([P, MC, R], BF, tag="snew")
            comb = states.tile([P, MC, R], BF, tag="comb")

            _red_uid = [0]

            def red_pair():
                _red_uid[0] += 1
                u = _red_uid[0]
                return (redps.tile([P, RH], F32, tag="red", name=f"red{u}a"),
                        redps.tile([P, RH], F32, tag="red", name=f"red{u}b"))

            def bias_ap(idx, mc):
                col = idx * MC + mc
                return bias_sb[:, col:col + 1]

            def wsc_ap(idx, mc):
                col = idx * MC + mc
                return wsc_sb[:, col:col + 1]

            def rsl(rh):
                return slice(rh * RH, (rh + 1) * RH)

            def mm_block(ps0, ps1, wt, src, kcn):
                """Accumulate one [2048->128] output block for both row
                chunks: DoubleRow over k-chunk pairs (plus a trailing single
                for odd kcn). kc-outer / rh-inner so the two matmuls sharing
                a stationary weight slice are adjacent."""
                npair = kcn // 2
                for kp in range(npair):
                    kc = 2 * kp
                    st = (kp == 0)
                    sp = (kc + 2 >= kcn)
                    nc.tensor.matmul(ps0[:], wt[:, kc:kc + 2, :],
                                     src[:, kc:kc + 2, rsl(0)],
                                     start=st, stop=sp, perf_mode=DR)
                    nc.tensor.matmul(ps1[:], wt[:, kc:kc + 2, :],
                                     src[:, kc:kc + 2, rsl(1)],
                                     start=st, stop=sp, perf_mode=DR)
                if kcn % 2:
                    kc = kcn - 1
                    nc.tensor.matmul(ps0[:], wt[:, kc, :], src[:, kc, rsl(0)],
                                     start=(kcn == 1), stop=True)
                    nc.tensor.matmul(ps1[:], wt[:, kc, :], src[:, kc, rsl(1)],
                                     start=(kcn == 1), stop=True)

            def term_pass(wname, kcn, src, evict, w0_tile=None, defer=2,
                          carry=(), ramp=False):
                """One linear term: stream weight blocks, accumulate psums,
                hand each [128, RH] psum chunk to `evict(mc, rh, ps)`.

                Evictions are emitted `defer` psum-groups late: the eviction
                chain (ACT relu -> DVE combine/square -> PE reduce-matmul)
                has ~1.5us of cross-engine latency, and emitting it inline
                makes the strict-FIFO PE queue stall on the reduce-matmul.

                The pass returns its last `defer` evictions instead of
                draining them: the CALLER either drains them inline (when
                the next pass's matmuls consume this pass's finale output)
                or hands them to the next pass as `carry` thunks, which run
                right after its first matmul block -- the ~2.2us of mains
                hides the drain's cross-engine chain, so the reduce-matmuls
                and the finale no longer serialize the pass boundary."""
                wd = w_d[wname]
                pending = []
                start_mc = 0
                if ramp:
                    # first RAMP blocks interleaved across 6 psums,
                    # consuming src k-chunks in ascending order: for passes
                    # whose src is being normalized by the immediately
                    # preceding finale, the PE consumes each chunk as the
                    # normalize mul produces it instead of stalling for all
                    # 16 chunks before block 0
                    wts, pss = [], []
                    for mc in range(RAMP):
                        wt = wpool.tile([P, kcn, P], F8, tag="w",
                                        name=f"rampw_{wname}_{mc}")
                        nc.sync.dma_start(out=wt[:], in_=wd[mc])
                        wts.append(wt)
                        pa = mmps.tile([P, RH], F32, tag="mm",
                                       name=f"rampp_{wname}_{mc}a")
                        pb = mmps.tile([P, RH], F32, tag="mm",
                                       name=f"rampp_{wname}_{mc}b")
                        pss.append((pa, pb))
                    for kp in range(kcn // 2):
                        kc = 2 * kp
                        st = (kp == 0)
                        sp = (kc + 2 >= kcn)
                        for mc in range(RAMP):
                            for rh in range(2):
                                nc.tensor.matmul(
                                    pss[mc][rh][:], wts[mc][:, kc:kc + 2, :],
                                    src[:, kc:kc + 2, rsl(rh)],
                                    start=st, stop=sp, perf_mode=DR)
                    for mc in range(RAMP):
                        pending.append((mc, 0, pss[mc][0]))
                        pending.append((mc, 1, pss[mc][1]))
                    start_mc = RAMP
                for mc in range(start_mc, MC):
                    if mc == 0 and w0_tile is not None:
                        wt = w0_tile
                    else:
                        wt = wpool.tile([P, kcn, P], F8, tag="w")
                        nc.sync.dma_start(out=wt[:], in_=wd[mc])
                    ps0 = mmps.tile([P, RH], F32, tag="mm")
                    ps1 = mmps.tile([P, RH], F32, tag="mm")
                    mm_block(ps0, ps1, wt, src, kcn)
                    if mc == start_mc:
                        for th in carry:
                            th()
                    pending.append((mc, 0, ps0))
                    pending.append((mc, 1, ps1))
                    while len(pending) > defer:
                        evict(*pending.pop(0))
                return evict, pending

            def finale_gated(tail, red, tgt):
                """Drain + finale for a pass whose output gates the very
                next pass: the rh0 inv chain is issued before the last rh1
                eviction drains, overlapping the two serial chains so the
                consumer's first (ramped) matmul unblocks ~1us sooner."""
                ev, pending = tail
                for it in pending[:-1]:
                    ev(*it)
                nr = small.tile([P, R], F32, tag="nr")
                inv = small.tile([P, R], F32, tag="inv")

                def chain(rh):
                    nc.vector.reciprocal_approx_fast(out=nr[:, rsl(rh)],
                                                     in_=red[rh][:])
                    nc.scalar.activation(
                        inv[:, rsl(rh)], nr[:, rsl(rh)],
                        mybir.ActivationFunctionType.Sqrt,
                        scale=SQSC * SSCALE * SSCALE)
                    for mc in range(2):
                        nc.vector.tensor_mul(tgt[:, mc, rsl(rh)],
                                             snew[:, mc, rsl(rh)],
                                             inv[:, rsl(rh)])

                chain(0)
                ev(*pending[-1])
                chain(1)
                for mc in range(2, MC):
                    nc.vector.tensor_mul(tgt[:, mc, :], snew[:, mc, :],
                                         inv[:])

            def drain(tail):
                ev, pending = tail
                for it in pending:
                    ev(*it)

            def mk_carry(tail, fin=None):
                ev, pending = tail
                ths = [lambda it=it: ev(*it) for it in pending]
                if fin is not None:
                    ths.append(fin)
                return ths

            _sqpair = {}

            def sq_and_reduce(mc, rh, red):
                """32*snew^2 in fp8e4 (x32 keeps the small squares above the
                fp8 subnormal floor; max ~120 < 240) into a 2-chunk pair
                tile; every odd mc issues one DoubleRow ones-matmul
                contracting both chunks -- half the reduce matmuls. red
                accumulates 32*sum(s^2), broadcast to all 128 partitions.
                On DVE (not ACT): keeps the ACT queue pure relu-evictions."""
                if mc % 2 == 0:
                    _sqpair[rh] = sqpool.tile([P, 2, RH], F8, tag="sq",
                                              name=f"sqp{rh}")
                t = _sqpair[rh]
                nc.vector.scalar_tensor_tensor(
                    t[:, mc % 2, :], snew[:, mc, rsl(rh)], SQSC,
                    snew[:, mc, rsl(rh)],
                    op0=mybir.AluOpType.mult, op1=mybir.AluOpType.mult)
                if mc % 2 == 1:
                    nc.tensor.matmul(red[rh][:], ones2[:], t[:],
                                     start=(mc == 1), stop=(mc == MC - 1),
                                     perf_mode=DR)

            def finale(red, tgt, goodness):
                """red[rh] holds 32*sum(s^2) per row, broadcast across all
                128 partitions. inv = SSCALE/sqrt(sum s^2): DVE fast
                reciprocal straight off the psum, then ACT sqrt with the
                scales folded into the input scale. eps dropped: n >= ~0.3
                always, so the relative effect is ~1e-4 (under the fp8
                noise floor). Normalize muls run full-R per mc so consumers'
                chunk k unblocks on mul #k."""
                if goodness:
                    for rh in range(2):
                        if goodness == "init":
                            nc.vector.tensor_copy(gacc[:, rsl(rh)],
                                                  red[rh][0:1, :])
                        else:
                            nc.vector.tensor_add(gacc[:, rsl(rh)],
                                                 gacc[:, rsl(rh)],
                                                 red[rh][0:1, :])
                if tgt is None:
                    return
                nr = small.tile([P, R], F32, tag="nr")
                inv = small.tile([P, R], F32, tag="inv")
                for rh in range(2):
                    nc.vector.reciprocal_approx_fast(out=nr[:, rsl(rh)],
                                                     in_=red[rh][:])
                    nc.scalar.activation(
                        inv[:, rsl(rh)], nr[:, rsl(rh)],
                        mybir.ActivationFunctionType.Sqrt,
                        scale=SQSC * SSCALE * SSCALE)
                    # the consumer's (ramped) first matmuls need chunks 0-1
                    # of this rh: emit them right after this rh's chain
                    # instead of behind both chains (full-R muls need all
                    # of inv)
                    for mc in range(2):
                        nc.vector.tensor_mul(tgt[:, mc, rsl(rh)],
                                             snew[:, mc, rsl(rh)],
                                             inv[:, rsl(rh)])
                for mc in range(2, MC):
                    nc.vector.tensor_mul(tgt[:, mc, :], snew[:, mc, :],
                                         inv[:])

            def evict_to(dst, bidx, widx):
                def ev(mc, rh, ps):
                    nc.scalar.activation(
                        dst[:, mc, rsl(rh)], ps[:],
                        mybir.ActivationFunctionType.Relu,
                        bias=bias_ap(bidx, mc), scale=wsc_ap(widx, mc))
                return ev

            def evict_add_comb(bidx, widx):
                def ev(mc, rh, ps):
                    e = epool.tile([P, RH], BF, tag="e")
                    nc.scalar.activation(
                        e[:], ps[:], mybir.ActivationFunctionType.Relu,
                        bias=bias_ap(bidx, mc), scale=wsc_ap(widx, mc))
                    nc.vector.tensor_add(comb[:, mc, rsl(rh)],
                                         e[:], comb[:, mc, rsl(rh)])
                return ev

            # ---- A = relu(hxn @ w1pre' + 0.7*b1pre), cached for all steps.
            # t0-n1 (snew = A + c1) is fused into the same pass so its
            # elementwise work overlaps the A matmuls chunk by chunk.
            red = red_pair()

            def ev_a(mc, rh, ps, red=red):
                nc.scalar.activation(
                    At[:, mc, rsl(rh)], ps[:],
                    mybir.ActivationFunctionType.Relu,
                    bias=bias_ap(B1PRE, mc), scale=wsc_ap(W1PRE, mc))
                nc.vector.tensor_scalar_add(
                    snew[:, mc, rsl(rh)], At[:, mc, rsl(rh)],
                    bias_ap(C1, mc))
                sq_and_reduce(mc, rh, red)

            # defer=4: the A pass produces chunks quickly, so the ~1.5us
            # eviction chain needs extra slack to stay hidden.
            # w2pre-t0 consumes s1 immediately -> drain + finale inline.
            finale_gated(term_pass("w1pre", KC1, hx, ev_a, w0_tile=w0,
                                    defer=2), red, s1)

            # ---- t0, n2 / n3: single pre-term + const.
            # t1-n1's post/self term passes are wedged between them: they
            # only need s2(t0)/s1(t0) and don't touch comb (the t0 updates
            # don't use it), so their matmuls fill t0's serial-chain tails.
            def ev_t0(red, cidx, bpre, widx):
                def ev(mc, rh, ps):
                    e = epool.tile([P, RH], BF, tag="e")
                    nc.scalar.activation(
                        e[:], ps[:], mybir.ActivationFunctionType.Relu,
                        bias=bias_ap(bpre, mc), scale=wsc_ap(widx, mc))
                    nc.vector.tensor_scalar_add(
                        snew[:, mc, rsl(rh)], e[:], bias_ap(cidx, mc))
                    sq_and_reduce(mc, rh, red)
                return ev

            red = red_pair()
            # w1post-t0 consumes s2 immediately -> drain + finale inline
            finale_gated(term_pass("w2pre", KC, s1,
                                    ev_t0(red, C2, B2PRE, W2PRE),
                                    ramp=True), red, s2)

            tail = term_pass("w1post", KC, s2,
                             evict_to(comb, B1POST, W1POST), ramp=True)
            tail = term_pass("w1self", KC, s1,
                             evict_add_comb(B1SELF, W1SELF),
                             carry=mk_carry(tail))

            red = red_pair()
            red_t0 = red
            tail = term_pass("w3pre", KC, s2, ev_t0(red, C3, B3PRE, W3PRE),
                             carry=mk_carry(tail), defer=3)
            drain(tail)
            finale(red, s3, None)

            def n1_combine(last):
                red = red_pair()
                for mc in range(MC):
                    for rh in range(2):
                        nc.vector.tensor_add(snew[:, mc, rsl(rh)],
                                             At[:, mc, rsl(rh)],
                                             comb[:, mc, rsl(rh)])
                        sq_and_reduce(mc, rh, red)
                # s1's consumer (w2pre) is 2+ passes away: defer the finale
                # into the next pass
                return lambda: finale(red, s1, "init" if last else None)

            # ---- t1 / t2
            for t in (1, 2):
                last = (t == 2)
                # n1 = A + relu(s2@w1post'+b) + relu(s1@w1self'+b)
                if t == 2:
                    tail = term_pass("w1post", KC, s2,
                                     evict_to(comb, B1POST, W1POST),
                                     carry=carry_in)
                    tail = term_pass("w1self", KC, s1,
                                     evict_add_comb(B1SELF, W1SELF),
                                     carry=mk_carry(tail))
                    drain(tail)
                fin_n1 = n1_combine(last)

                # n2 = relu(s1new@w2pre') + relu(s3@w2post') + relu(s2@w2self')
                tail = term_pass("w2post", KC, s3,
                                 evict_to(comb, B2POST, W2POST),
                                 carry=[fin_n1])
                tail = term_pass("w2self", KC, s2,
                                 evict_add_comb(B2SELF, W2SELF),
                                 carry=mk_carry(tail))
                red = red_pair()

                def ev_n2(mc, rh, ps, red=red):
                    e = epool.tile([P, RH], BF, tag="e")
                    nc.scalar.activation(
                        e[:], ps[:], mybir.ActivationFunctionType.Relu,
                        bias=bias_ap(B2PRE, mc), scale=wsc_ap(W2PRE, mc))
                    nc.vector.tensor_add(snew[:, mc, rsl(rh)],
                                         e[:], comb[:, mc, rsl(rh)])
                    sq_and_reduce(mc, rh, red)

                tail = term_pass("w2pre", KC, s1, ev_n2,
                                 carry=mk_carry(tail), defer=3)
                fin_n2 = (lambda red=red, g=("add" if last else None):
                          finale(red, s2, g))

                # n3 = relu(s2new@w3pre') + c3p + relu(s3@w3self')
                tail = term_pass("w3self", KC, s3,
                                 evict_to(comb, B3SELF, W3SELF),
                                 carry=mk_carry(tail, fin_n2))
                red = red_pair()

                def ev_n3(mc, rh, ps, red=red):
                    e = epool.tile([P, RH], BF, tag="e")
                    nc.scalar.activation(
                        e[:], ps[:], mybir.ActivationFunctionType.Relu,
                        bias=bias_ap(B3PRE, mc), scale=wsc_ap(W3PRE, mc))
                    nc.vector.scalar_tensor_tensor(
                        snew[:, mc, rsl(rh)], e[:], bias_ap(C3P, mc),
                        comb[:, mc, rsl(rh)],
                        op0=mybir.AluOpType.add, op1=mybir.AluOpType.add)
                    sq_and_reduce(mc, rh, red)

                tail = term_pass("w3pre", KC, s2, ev_n3,
                                 carry=mk_carry(tail), defer=3)
                if last:
                    drain(tail)
                    finale(red, None, "add")
                else:
                    carry_in = mk_carry(
                        tail, (lambda red=red: finale(red, s3, None)))

            # ---- goodness out: gacc holds 32*sum(s^2); g = gacc/(32*2048)
            gout = consts.tile([1, R], F32, tag="gout")
            nc.scalar.mul(gout[:], gacc[:], 1.0 / (H * SQSC))
            nc.sync.dma_start(out=g_d[:], in_=gout[:])

    nc.compile()
    return nc


def _quant_weight(w, scale, kcn):
    """[2048, d_in] float32 -> ([MC, P, kcn, P] fp8e4 blocked for linear DMA,
    [128, 16] per-output-row descale columns).

    host_w[mc, p, kc, m] = alpha_row[mc*128+m] * scale * W[mc*128+m, kc*128+p]
    with alpha_row a power of 2 chosen so each row's absmax lands in
    (112, 224] (fp8e4 max normal 240). Descale col = 1/(128*alpha_row)."""
    w = np.asarray(w, dtype=np.float32) * scale
    din = w.shape[1]
    absmax = np.abs(w).max(axis=1)
    absmax = np.maximum(absmax, 1e-30)
    alpha = np.exp2(np.floor(np.log2(224.0 / absmax)))
    wq = w * alpha[:, None]
    if din < kcn * P:
        wq = np.pad(wq, ((0, 0), (0, kcn * P - din)))
    blk = wq.reshape(MC, P, kcn, P).transpose(0, 3, 2, 1)
    blk = np.ascontiguousarray(blk.astype(NPF8))
    descale = (1.0 / (SSCALE * alpha)).astype(np.float32).reshape(MC, P).T
    return blk, np.ascontiguousarray(descale)


def _col(v):
    """[2048] -> [128, 16] (partition-major bias layout)."""
    return np.asarray(v, dtype=np.float32).reshape(MC, P).T


def prepare_inputs(inputs):
    """Host prep: overlay+normalize Hx, quantize/block weights, pack biases.
    Returns (shared_map, per_core_hx list)."""
    x = np.asarray(inputs["x"], dtype=np.float32)
    mx = x.max()
    base = x.copy()
    base[:, :NL] = 0.0
    hx = np.tile(base[None, :, :], (NL, 1, 1))
    for l in range(NL):
        hx[l, :, l] = mx
    hx = hx.reshape(ROWS, D_IN)
    n = np.linalg.norm(hx, axis=1, keepdims=True)
    hxn = (hx / (n + EPS)) * SSCALE
    hxn = np.pad(hxn, ((0, 0), (0, D_IN_PAD - D_IN)))

    per_core_hx = []
    for c in range(N_CORES):
        h = hxn[c * R:(c + 1) * R].T            # [896, 640]
        h = h.reshape(KC1, P, R).transpose(1, 0, 2)
        per_core_hx.append(np.ascontiguousarray(h.astype(NPF8)))

    wspec = [
        ("w1pre", "w1_pre", 0.7, KC1, W1PRE),
        ("w1post", "w1_post", 0.7, KC, W1POST),
        ("w1self", "w1_self", 0.3, KC, W1SELF),
        ("w2pre", "w2_pre", 0.7, KC, W2PRE),
        ("w2post", "w2_post", 0.7, KC, W2POST),
        ("w2self", "w2_self", 0.3, KC, W2SELF),
        ("w3pre", "w3_pre", 0.7, KC, W3PRE),
        ("w3self", "w3_self", 0.3, KC, W3SELF),
    ]
    shared = {}
    wscales = np.empty((P, NW * MC), dtype=np.float32)
    for dname, iname, sc, kcn, widx in wspec:
        blk, desc = _quant_weight(inputs[iname], sc, kcn)
        shared[dname] = blk
        wscales[:, widx * MC:(widx + 1) * MC] = desc
    shared["wscales"] = np.ascontiguousarray(wscales)

    relu = lambda a: np.maximum(np.asarray(a, dtype=np.float32), 0.0)

    cols = np.empty((P, NBIAS * MC), dtype=np.float32)
    vals = {
        B1PRE: 0.7 * np.asarray(inputs["b1_pre"], np.float32),
        B1POST: 0.7 * np.asarray(inputs["b1_post"], np.float32),
        B1SELF: 0.3 * np.asarray(inputs["b1_self"], np.float32),
        B2PRE: 0.7 * np.asarray(inputs["b2_pre"], np.float32),
        B2POST: 0.7 * np.asarray(inputs["b2_post"], np.float32),
        B2SELF: 0.3 * np.asarray(inputs["b2_self"], np.float32),
        B3PRE: 0.7 * np.asarray(inputs["b3_pre"], np.float32),
        B3SELF: 0.3 * np.asarray(inputs["b3_self"], np.float32),
        C1: 0.7 * relu(inputs["b1_post"]) + 0.3 * relu(inputs["b1_self"]),
        C2: 0.7 * relu(inputs["b2_post"]) + 0.3 * relu(inputs["b2_self"]),
        C3: 0.7 * relu(inputs["b3_post"]) + 0.3 * relu(inputs["b3_self"]),
        C3P: 0.7 * relu(inputs["b3_post"]),
    }
    for idx, v in vals.items():
        cols[:, idx * MC:(idx + 1) * MC] = _col(v)
    shared["biases"] = np.ascontiguousarray(cols)

    return shared, per_core_hx


def run(inputs, trace=False):
    shared, per_core_hx = prepare_inputs(inputs)
    if "nc" not in _NC_CACHE:
        _NC_CACHE["nc"] = _build_nc()
    nc = _NC_CACHE["nc"]
    in_maps = [dict(shared, hxn=per_core_hx[c]) for c in range(N_CORES)]
    res = run_bass_kernel_spmd(nc, in_maps, core_ids=list(range(N_CORES)),
                               trace=trace)
    g = np.concatenate([res.results[c]["g"][0] for c in range(N_CORES)])
    out = g.reshape(NL, B).T.astype(np.float32)
    return np.ascontiguousarray(out), res


def kernel(**inputs):
    out, _ = run(inputs, trace=False)
    return out
